# revision 1
# baseline (speedup 1.0000x reference)
"""Causal self-attention (B=2, T=2048, C=1024, H=16) on 8 TRN2 NeuronCores.

Sharding: core = (batch b, head-group hg) with b in {0,1}, hg in {0..3};
each core computes Q/K/V projections and attention for its 4 heads on its
batch, plus the row-parallel slice of the output projection. The host sums
the 4 per-core partial projections per batch and adds the output bias.

Device algorithm (all layouts transposed so softmax needs no on-chip
transposes):
  - Q^T, K^T [dd, t] and V [t, dd] via fp32r matmuls (contraction over C).
  - S^T[s, t] = K^T.T-free matmul pair, 2 heads row-packed in the 128-wide
    PE array (K=64 each at row offsets 0/64).
  - exp on ScalarE straight out of PSUM (scale=1/sqrt(d) folded in); causal
    masking = one additive 128x128 band on diagonal blocks + trimming the
    AV matmul's moving range; softmax denominators from an all-ones column
    appended to V (M=65 matmul); normalization deferred to after AV.
  - y_partial[t, e] = O^T.T @ Wp^T slice, accumulated over the 2 dd-chunks.
"""

import math
from functools import lru_cache

import ml_dtypes
import numpy as np

import concourse.bass as bass
import concourse.mybir as mybir
from concourse import bacc
import concourse.tile as tile
from concourse import bass_utils

F32 = mybir.dt.float32
F32R = mybir.dt.float32r
BF16 = mybir.dt.bfloat16
EXP = mybir.ActivationFunctionType.Exp

B, T, C, H = 2, 2048, 1024, 16
NCORES = 8
NH = 4            # heads per core
D = C // H        # 64
DD = NH * D       # 256 channels per core
P = 128
TG = 512          # t-group width (matmul moving dim)
NG = T // TG      # 4
NT = T // P       # 16 s-chunks
CCH = C // P      # 8 contraction chunks
NEG = -8.0e6      # pre-scale additive mask; *0.125 = -1e6 like the reference

LAST_RESULTS = None  # BassKernelResults of the most recent run (for test.py)


def build_program(apply_kbias: bool, general_mask: bool) -> bass.Bass:
    nc = bacc.Bacc("TRN2", target_bir_lowering=False, debug=False,
                   enable_asserts=False)

    xT = nc.dram_tensor("xT", [C, T], BF16, kind="ExternalInput").ap()
    wqT = nc.dram_tensor("wqT", [C, DD], BF16, kind="ExternalInput").ap()
    wkT = nc.dram_tensor("wkT", [C, DD], BF16, kind="ExternalInput").ap()
    wvT = nc.dram_tensor("wvT", [C, DD], BF16, kind="ExternalInput").ap()
    wpT = nc.dram_tensor("wpT", [DD, C], BF16, kind="ExternalInput").ap()
    bqk = nc.dram_tensor("bqk", [P, 4], F32, kind="ExternalInput").ap()
    bv_in = nc.dram_tensor("bv_sb", [P, DD], F32, kind="ExternalInput").ap()
    kbias_in = None
    if apply_kbias:
        kbias_in = nc.dram_tensor("kbias", [P, NT], F32, kind="ExternalInput").ap()
    band_in = maskT = None
    if general_mask:
        maskT = nc.dram_tensor("maskT", [T, T], F32, kind="ExternalInput").ap()
    else:
        band_in = nc.dram_tensor("band", [P, P], F32, kind="ExternalInput").ap()
    yp = nc.dram_tensor("yp", [T, C], F32, kind="ExternalOutput").ap()
    # DRAM scratch used to broadcast softmax reciprocal rows across
    # partitions (DMA from DRAM may use a 0-step partition dim; SBUF may not)
    rcd = nc.dram_tensor("rcd", [8, 2 * TG], F32, kind="Internal").ap()

    with tile.TileContext(nc) as tc:
        with tc.tile_pool(name="wts", bufs=1) as wts, \
             tc.tile_pool(name="xtp", bufs=1) as xtp, \
             tc.tile_pool(name="qkv", bufs=1) as qkv, \
             tc.tile_pool(name="otp", bufs=1) as otp, \
             tc.tile_pool(name="ptp", bufs=2) as ptp, \
             tc.tile_pool(name="asb", bufs=4) as asbp, \
             tc.tile_pool(name="rcp", bufs=2) as rcp, \
             tc.tile_pool(name="bcp", bufs=2) as bcp, \
             tc.tile_pool(name="tmp", bufs=2) as tmpp, \
             tc.tile_pool(name="ysb", bufs=2) as ysbp, \
             tc.tile_pool(name="mkp", bufs=2) as mkp, \
             tc.tile_pool(name="stp", bufs=2, space="PSUM") as stp, \
             tc.tile_pool(name="avp", bufs=2, space="PSUM") as avp, \
             tc.tile_pool(name="mmp", bufs=2, space="PSUM") as mmp:

            # Pre-load the one ACT table set containing BOTH Exp and Ln so
            # the act-table pass doesn't thrash between per-function sets
            # (measured 17 TABLE_LOADs / 22us without this).
            from concourse.hw_specs import get_activation_tables
            tables = get_activation_tables(nc.m.arch)
            combined_id = list(tables).index("natural_log_exp_and_others")
            nc.scalar.add_instruction(mybir.InstLoadActFuncSet(
                name=nc.get_next_instruction_name(), ins=[], outs=[],
                act_func_set_id=combined_id))

            # ---- input DMAs ----
            wq = [wts.tile([P, DD], BF16, name=f"wq{c}") for c in range(CCH)]
            wk = [wts.tile([P, DD], BF16, name=f"wk{c}") for c in range(CCH)]
            wv = [wts.tile([P, DD], BF16, name=f"wv{c}") for c in range(CCH)]
            xt = [xtp.tile([P, T], BF16, name=f"xt{c}") for c in range(CCH)]
            bqk_t = wts.tile([P, 4], F32, name="bqk_t")
            nc.sync.dma_start(out=bqk_t, in_=bqk)
            # x + K/Q weights first: the projection chains consume xt[c] in
            # order, so interleave per-chunk to start the PE earliest.
            for c in range(CCH):
                nc.sync.dma_start(out=wk[c], in_=wkT[c * P:(c + 1) * P, :])
                nc.sync.dma_start(out=wq[c], in_=wqT[c * P:(c + 1) * P, :])
                nc.sync.dma_start(out=xt[c], in_=xT[c * P:(c + 1) * P, :])
            for c in range(CCH):
                nc.sync.dma_start(out=wv[c], in_=wvT[c * P:(c + 1) * P, :])
            wp = [wts.tile([P, C], BF16, name=f"wp{i}") for i in range(2)]
            for i in range(2):
                nc.sync.dma_start(out=wp[i], in_=wpT[i * P:(i + 1) * P, :])
            bv_sb = wts.tile([P, DD], F32, name="bv_t")
            nc.sync.dma_start(out=bv_sb, in_=bv_in)
            if band_in is not None:
                band_t = wts.tile([P, P], F32, name="band_t")
                nc.sync.dma_start(out=band_t, in_=band_in)
            if kbias_in is not None:
                kbias_t = wts.tile([P, NT], F32, name="kbias_t")
                nc.sync.dma_start(out=kbias_t, in_=kbias_in)

            qt = [qkv.tile([P, T], BF16, name=f"qt{i}") for i in range(2)]
            kt = [qkv.tile([P, T], BF16, name=f"kt{i}") for i in range(2)]
            vaug = [qkv.tile([P, NH * (D + 1)], BF16, name=f"vaug{j}")
                    for j in range(NT)]
            ot = [otp.tile([P, T], BF16, name=f"ot{i}") for i in range(2)]

            # ---- QKV projections ----
            # During QKV the attention PSUM pools are idle; rotating chains
            # across all three pools lets 6 accumulation chains run while
            # xT streams in, instead of 2.
            def qkv_ps():
                return mmp.tile([P, TG], F32, name="mm", tag="mm")

            def qk_proj(i, dst, w, bias_col):
                for tg in range(NG):
                    ps = qkv_ps()
                    for c in range(CCH):
                        nc.tensor.matmul(
                            ps,
                            lhsT=(w[c][:, i * P:(i + 1) * P]),
                            rhs=(xt[c][:, tg * TG:(tg + 1) * TG]),
                            start=(c == 0), stop=(c == CCH - 1))
                    nc.vector.tensor_scalar_add(
                        dst[:, tg * TG:(tg + 1) * TG], ps,
                        bqk_t[:, bias_col:bias_col + 1])

            def v_proj(j):
                ps = qkv_ps()
                for c in range(CCH):
                    nc.tensor.matmul(
                        ps[:, :DD],
                        lhsT=(xt[c][:, j * P:(j + 1) * P]),
                        rhs=(wv[c]),
                        start=(c == 0), stop=(c == CCH - 1))
                vview = vaug[j].rearrange("p (h x) -> p h x", h=NH)
                # ones column (softmax denominator row): in0*0 + 1 -> f32r
                nc.vector.tensor_scalar(
                    vview[:, :, D:D + 1],
                    bv_sb.rearrange("p (h x) -> p h x", h=NH)[:, :, 0:1],
                    0.0, 1.0,
                    mybir.AluOpType.mult, mybir.AluOpType.add)
                nc.vector.tensor_add(
                    vview[:, :, 0:D],
                    ps[:, :DD].rearrange("p (h x) -> p h x", h=NH),
                    bv_sb.rearrange("p (h x) -> p h x", h=NH))


            # ---- attention ----
            def attn_block(i, g):
                # causal: only s-chunks on/below the diagonal contribute.
                # general mask: every s-chunk may contribute.
                nj = NT if general_mask else 4 * g + 4
                av = [avp.tile([P, TG], F32, name="av", tag="av")
                      for _ in range(2)]
                for j in range(nj):
                    r = j - 4 * g  # >=0 on diagonal blocks
                    st = stp.tile([P, 2 * TG], F32, name="st", tag="st")
                    for h in range(2):
                        nc.tensor.matmul(
                            st[:, h * TG:(h + 1) * TG],
                            lhsT=(kt[i][64 * h:64 * h + 64,
                                             j * P:(j + 1) * P]),
                            rhs=(qt[i][64 * h:64 * h + 64,
                                            g * TG:(g + 1) * TG]),
                            start=True, stop=True,
                            tile_position=(64 * h, 0))
                    if general_mask:
                        mk = mkp.tile([P, TG], F32, name="mk", tag="mk")
                        nc.sync.dma_start(
                            out=mk,
                            in_=maskT[j * P:(j + 1) * P, g * TG:(g + 1) * TG])
                        for h in range(2):
                            nc.vector.tensor_add(
                                st[:, h * TG:(h + 1) * TG],
                                st[:, h * TG:(h + 1) * TG], mk)
                    elif r >= 0:
                        for h in range(2):
                            sl = slice(h * TG + r * P, h * TG + (r + 1) * P)
                            nc.vector.tensor_add(st[:, sl], st[:, sl], band_t)
                    if apply_kbias:
                        for h in range(2):
                            nc.vector.tensor_scalar_add(
                                st[:, h * TG:(h + 1) * TG],
                                st[:, h * TG:(h + 1) * TG],
                                kbias_t[:, j:j + 1])
                    pt = ptp.tile([P, 2 * TG], BF16, name="pt", tag="pt")
                    nc.scalar.activation(pt, st, EXP, scale=1.0 / math.sqrt(D))
                    trim = r * P if (r > 0 and not general_mask) else 0
                    for h in range(2):
                        nc.tensor.matmul(
                            av[h][0:D + 1, trim:TG],
                            lhsT=(vaug[j][:, (2 * i + h) * (D + 1):
                                               (2 * i + h + 1) * (D + 1)]),
                            rhs=(pt[:, h * TG + trim:(h + 1) * TG]),
                            start=(j == 0), stop=(j == nj - 1),
                            skip_group_check=True)
                # one fast PSUM->SBUF copy per head releases the accumulator
                # bank; reciprocal = exp(-ln(sums)) on ScalarE (keeps the DVE
                # FIFO free), broadcast across partitions via a DRAM bounce.
                slot = i * NG + g
                asb = asbp.tile([D + 1, 2 * TG], F32, name="asb", tag="asb")
                nc.vector.tensor_copy(asb[:, 0:TG], av[0][0:D + 1, :])
                nc.vector.tensor_copy(asb[:, TG:2 * TG], av[1][0:D + 1, :])
                rc = rcp.tile([P, 2 * TG], F32, name="rc", tag="rc")
                nc.scalar.activation(
                    rc[D:D + 1, :], asb[D:D + 1, :],
                    mybir.ActivationFunctionType.Ln)
                nc.scalar.activation(
                    rc[D:D + 1, :], rc[D:D + 1, :],
                    EXP, scale=-1.0)
                nc.sync.dma_start(out=rcd[slot], in_=rc[D:D + 1, :])
                bc = bcp.tile([P, 2 * TG], F32, name="bc", tag="bc")
                bcast_src = bass.AP(
                    tensor=rcd.tensor, offset=rcd[slot].offset,
                    ap=[[0, D]] + list(rcd[slot].ap))
                nc.sync.dma_start(out=bc[0:D, :], in_=bcast_src)
                nc.vector.tensor_mul(
                    ot[i][0:D, g * TG:(g + 1) * TG],
                    asb[0:D, 0:TG], bc[0:D, 0:TG])
                tm = tmpp.tile([P, TG], BF16, name="tm", tag="tm")
                nc.vector.tensor_mul(tm[0:D, :], asb[0:D, TG:2 * TG],
                                     bc[0:D, TG:2 * TG])
                nc.sync.dma_start(
                    out=ot[i][64:128, g * TG:(g + 1) * TG],
                    in_=tm[0:D, :])

            def proj_block(tt, ec):
                ps = mmp.tile([P, TG], F32, name="mm", tag="mm")
                for i in range(2):
                    nc.tensor.matmul(
                        ps,
                        lhsT=(ot[i][:, tt * P:(tt + 1) * P]),
                        rhs=(wp[i][:, ec * TG:(ec + 1) * TG]),
                        start=(i == 0), stop=(i == 1))
                ysb = ysbp.tile([P, TG], F32, name="ysb", tag="ysb")
                nc.vector.tensor_copy(ysb, ps)
                nc.sync.dma_start(
                    out=yp[tt * P:(tt + 1) * P, ec * TG:(ec + 1) * TG],
                    in_=ysb)

            # Emission order drives Tile's scheduling priority: pair-0
            # inputs, then pair-0's biggest attention block interleaved with
            # pair-1's projections (keeps ScalarE exp-ing while the PE runs
            # QKV), then the rest in descending-g order so the last block
            # (and the projections serialized behind it) is the smallest.
            qk_proj(0, kt[0], wk, 2)
            qk_proj(0, qt[0], wq, 0)
            for j in range(NT):
                v_proj(j)
            attn_block(0, NG - 1)
            qk_proj(1, kt[1], wk, 3)
            qk_proj(1, qt[1], wq, 1)
            for g in range(NG - 1, -1, -1):
                if g != NG - 1:
                    attn_block(0, g)
                attn_block(1, g)
                for tt in range(4 * g, 4 * g + 4):
                    for ec in range(2):
                        proj_block(tt, ec)

    nc.compile()
    return nc


@lru_cache(maxsize=4)
def _program(apply_kbias: bool, general_mask: bool) -> bass.Bass:
    return build_program(apply_kbias, general_mask)


def _host_prep(inputs):
    x = np.asarray(inputs["x"], np.float32)
    Wq = np.asarray(inputs["Wq"], np.float32)
    bq = np.asarray(inputs["bq"], np.float32)
    Wk = np.asarray(inputs["Wk"], np.float32)
    bk = np.asarray(inputs["bk"], np.float32)
    Wv = np.asarray(inputs["Wv"], np.float32)
    bv = np.asarray(inputs["bv"], np.float32)
    Wp = np.asarray(inputs["Wp"], np.float32)
    attn_mask = np.asarray(inputs["attn_mask"])
    valid = np.asarray(inputs["valid_input_mask"])

    tril = np.tril(np.ones((T, T), attn_mask.dtype))
    causal = all(np.array_equal(attn_mask[b], tril) for b in range(B))
    kbias_all = (valid.astype(np.float32) - 1.0) * 1e6  # [B, T]
    apply_kbias = bool((valid == 0).any())

    band = np.where(np.arange(P)[:, None] <= np.arange(P)[None, :],
                    np.float32(0.0), np.float32(NEG))

    in_maps = []
    for core in range(NCORES):
        b, hg = divmod(core, 4)
        sl = slice(hg * DD, (hg + 1) * DD)
        m = {
            "xT": np.ascontiguousarray(x[b].T).astype(ml_dtypes.bfloat16),
            "wqT": np.ascontiguousarray(Wq[sl, :].T).astype(ml_dtypes.bfloat16),
            "wkT": np.ascontiguousarray(Wk[sl, :].T).astype(ml_dtypes.bfloat16),
            "wvT": np.ascontiguousarray(Wv[sl, :].T).astype(ml_dtypes.bfloat16),
            "wpT": np.ascontiguousarray(Wp[:, sl].T).astype(ml_dtypes.bfloat16),
            "bqk": np.ascontiguousarray(
                np.stack([bq[sl][:P], bq[sl][P:], bk[sl][:P], bk[sl][P:]], 1)),
            "bv_sb": np.ascontiguousarray(np.tile(bv[sl], (P, 1))),
        }
        if apply_kbias:
            m["kbias"] = np.ascontiguousarray(kbias_all[b].reshape(NT, P).T)
        if not causal:
            m["maskT"] = np.ascontiguousarray(
                (attn_mask[b].T.astype(np.float32) - 1.0) * (-NEG))
        else:
            m["band"] = band
        in_maps.append(m)
    return in_maps, apply_kbias, causal


def _run(inputs, trace=False, trace_cores=None):
    global LAST_RESULTS
    in_maps, apply_kbias, causal = _host_prep(inputs)
    nc = _program(apply_kbias, not causal)
    res = bass_utils.run_bass_kernel_spmd(
        nc, in_maps, core_ids=list(range(NCORES)), trace=trace,
        trace_cores=trace_cores)
    LAST_RESULTS = res

    bp = np.asarray(inputs["bp"], np.float32)
    y = np.zeros((B, T, C), np.float32)
    for core in range(NCORES):
        y[core // 4] += res.results[core]["yp"]
    y += bp[None, None, :]
    return y


def kernel(**inputs) -> np.ndarray:
    return _run(inputs)



# revision 3
# speedup vs baseline: 1.0633x; 1.0633x over previous
"""Causal self-attention (B=2, T=2048, C=1024, H=16) on 8 TRN2 NeuronCores.

Sharding: core = (batch b, head-group hg) with b in {0,1}, hg in {0..3};
each core computes Q/K/V projections and attention for its 4 heads on its
batch, plus the row-parallel slice of the output projection. The host sums
the 4 per-core partial projections per batch (bf16) and adds the output bias.

Device algorithm (all layouts transposed so softmax needs no on-chip
transposes):
  - x streamed t-group-major ([c, tg] tiles via one 3D-AP DMA per tg) so the
    K/Q projection chains start ~4us after launch; a burst of tiny warm-up
    matmuls releases the HAM clock throttle before the first real chain.
  - Q^T, K^T [dd, t] and V [t, dd] via bf16 matmul chains (contraction over C).
  - S^T[s, t]: 2 heads row-packed in the PE array (K=64 at row offsets 0/64).
  - exp on ScalarE straight out of PSUM (scale=1/sqrt(d) folded in); causal
    masking = one additive 128x128 band on diagonal blocks + trimming the
    AV matmul's moving range; softmax denominators from an all-ones column
    appended to V (M=65 matmul); normalization deferred to after AV.
  - softmax reciprocals: denominator row bounced through DRAM into a
    [128, 8] partition-major tile, one DVE reciprocal op (ScalarE runs
    exps only), bounced back and broadcast-read for the normalize muls.
  - Emission interleaves projection/QKV chains into the attention blocks'
    spare PE slots (attention is ScalarE-exp paced) so the PE never idles.
  - y_partial[t, e] bf16 out; host sums partials in f32.
"""

import math
from collections import deque
from functools import lru_cache

import ml_dtypes
import numpy as np

import concourse.bass as bass
import concourse.mybir as mybir
from concourse import bacc
import concourse.tile as tile
from concourse import bass_utils

F32 = mybir.dt.float32
BF16 = mybir.dt.bfloat16
EXP = mybir.ActivationFunctionType.Exp

B, T, C, H = 2, 2048, 1024, 16
NCORES = 8
NH = 4            # heads per core
D = C // H        # 64
DD = NH * D       # 256 channels per core
P = 128
TG = 512          # t-group width (matmul moving dim)
NG = T // TG      # 4
NT = T // P       # 16 s-chunks
CCH = C // P      # 8 contraction chunks
NEG = -8.0e6      # pre-scale additive mask; *0.125 = -1e6 like the reference

LAST_RESULTS = None  # BassKernelResults of the most recent run (for test.py)


class FillerQueue:
    """PE work units interleaved into the attention blocks' spare slots."""

    def __init__(self):
        self.q = deque()

    def add(self, units):
        self.q.extend(units)

    def pump(self, n=1):
        for _ in range(n):
            if not self.q:
                return
            self.q.popleft()()

    def flush(self):
        while self.q:
            self.q.popleft()()


def build_program(apply_kbias: bool, general_mask: bool) -> bass.Bass:
    nc = bacc.Bacc("TRN2", target_bir_lowering=False, debug=False,
                   enable_asserts=False)

    xT = nc.dram_tensor("xT", [C, T], BF16, kind="ExternalInput").ap()
    wqT = nc.dram_tensor("wqT", [C, DD], BF16, kind="ExternalInput").ap()
    wkT = nc.dram_tensor("wkT", [C, DD], BF16, kind="ExternalInput").ap()
    wvT = nc.dram_tensor("wvT", [C, DD], BF16, kind="ExternalInput").ap()
    wpT = nc.dram_tensor("wpT", [DD, C], BF16, kind="ExternalInput").ap()
    bqk = nc.dram_tensor("bqk", [P, 4], F32, kind="ExternalInput").ap()
    bv_in = nc.dram_tensor("bv_sb", [P, DD], F32, kind="ExternalInput").ap()
    kbias_in = None
    if apply_kbias:
        kbias_in = nc.dram_tensor("kbias", [P, NT], F32, kind="ExternalInput").ap()
    band_in = maskT = None
    if general_mask:
        maskT = nc.dram_tensor("maskT", [T, T], F32, kind="ExternalInput").ap()
    else:
        band_in = nc.dram_tensor("band", [P, P], F32, kind="ExternalInput").ap()
    yp = nc.dram_tensor("yp", [T, C], BF16, kind="ExternalOutput").ap()
    # DRAM bounce buffers for the softmax denominators: raw rows land in
    # rcd_raw, get re-read [128, 8] partition-major (contiguous 8-elem lines),
    # reciprocated on DVE, written back t-major to rcd_rcp, then broadcast
    # across partitions (DMA from DRAM may use a 0-step partition dim).
    rcd_raw = nc.dram_tensor("rcd_raw", [2 * NG, 2 * TG], BF16, kind="Internal").ap()
    rcd_rcp = nc.dram_tensor("rcd_rcp", [2 * NG, 2 * TG], BF16, kind="Internal").ap()

    with tile.TileContext(nc) as tc:
        with tc.tile_pool(name="wts", bufs=1) as wts, \
             tc.tile_pool(name="xtp", bufs=1) as xtp, \
             tc.tile_pool(name="qkv", bufs=1) as qkv, \
             tc.tile_pool(name="otp", bufs=1) as otp, \
             tc.tile_pool(name="ptp", bufs=2) as ptp, \
             tc.tile_pool(name="asb", bufs=4) as asbp, \
             tc.tile_pool(name="rtp", bufs=2) as rtp, \
             tc.tile_pool(name="bcp", bufs=2) as bcp, \
             tc.tile_pool(name="tmp", bufs=2) as tmpp, \
             tc.tile_pool(name="ysb", bufs=2) as ysbp, \
             tc.tile_pool(name="mkp", bufs=2) as mkp, \
             tc.tile_pool(name="stp", bufs=2, space="PSUM") as stp, \
             tc.tile_pool(name="avp", bufs=2, space="PSUM") as avp, \
             tc.tile_pool(name="mmp", bufs=2, space="PSUM") as mmp:

            # Only Exp (and friends) are needed; preload so the act-table
            # DMA overlaps the input DMAs instead of stalling the first exp.
            from concourse.hw_specs import get_activation_tables
            tables = get_activation_tables(nc.m.arch)
            set_id = list(tables).index("exp_and_others")
            nc.scalar.add_instruction(mybir.InstLoadActFuncSet(
                name=nc.get_next_instruction_name(), ins=[], outs=[],
                act_func_set_id=set_id))

            # ---- input DMAs ----
            bqk_t = wts.tile([P, 4], F32, name="bqk_t")
            nc.sync.dma_start(out=bqk_t, in_=bqk)

            # weights gathered c-major into single wide tiles (1 DMA each):
            # w[p, c*DD+q] = wT[c*P+p, q]
            def w_src(wT):
                return bass.AP(tensor=wT.tensor, offset=wT.offset,
                               ap=[[DD, P], [P * DD, CCH], [1, DD]])

            wkall = wts.tile([P, CCH * DD], BF16, name="wkall")
            wqall = wts.tile([P, CCH * DD], BF16, name="wqall")
            wvall = wts.tile([P, CCH * DD], BF16, name="wvall")
            # x gathered t-group-major: xtg[tg][p, c*TG+u] = xT[c*P+p, tg*TG+u]
            xtg = [xtp.tile([P, CCH * TG], BF16, name=f"xtg{t_}")
                   for t_ in range(NG)]

            def x_src(tg):
                return bass.AP(tensor=xT.tensor, offset=tg * TG,
                               ap=[[T, P], [P * T, CCH], [1, TG]])

            nc.sync.dma_start(out=wkall, in_=w_src(wkT))
            nc.sync.dma_start(out=xtg[0], in_=x_src(0))
            nc.sync.dma_start(out=wqall, in_=w_src(wqT))
            for t_ in range(1, NG):
                nc.sync.dma_start(out=xtg[t_], in_=x_src(t_))
            nc.sync.dma_start(out=wvall, in_=w_src(wvT))
            bv_sb = wts.tile([P, DD], F32, name="bv_t")
            nc.sync.dma_start(out=bv_sb, in_=bv_in)
            if band_in is not None:
                band_t = wts.tile([P, P], F32, name="band_t")
                nc.sync.dma_start(out=band_t, in_=band_in)
            if kbias_in is not None:
                kbias_t = wts.tile([P, NT], F32, name="kbias_t")
                nc.sync.dma_start(out=kbias_t, in_=kbias_in)
            wp = [wts.tile([P, C], BF16, name=f"wp{i}") for i in range(2)]
            for i in range(2):
                nc.sync.dma_start(out=wp[i], in_=wpT[i * P:(i + 1) * P, :])

            qt = [qkv.tile([P, T], BF16, name=f"qt{i}") for i in range(2)]
            kt = [qkv.tile([P, T], BF16, name=f"kt{i}") for i in range(2)]
            vaug = [qkv.tile([P, NH * (D + 1)], BF16, name=f"vaug{j}")
                    for j in range(NT)]
            ot = [otp.tile([P, T], BF16, name=f"ot{i}") for i in range(2)]

            # ---- HAM warm-up: ~3us of tiny matmuls so the PE clock is at
            # 8/8 by the time the first projection chain lands.
            wps = mmp.tile([P, TG], F32, name="mm", tag="mm")
            for _ in range(64):
                nc.tensor.matmul(wps[0:1, 0:1], lhsT=bqk_t[:, 0:1],
                                 rhs=bqk_t[:, 0:1], start=True, stop=True)

            # ---- QKV chain units ----
            def wsl(wall, c, iw):
                return wall[:, c * DD + iw * P: c * DD + iw * P + P]

            def xsl(tg, c):
                return xtg[tg][:, c * TG:(c + 1) * TG]

            def xvsl(j, c):
                tg, u = divmod(j, NG)
                return xtg[tg][:, c * TG + u * P: c * TG + u * P + P]

            def qk_chain_units(iw, tg):
                """K then Q projection chain for dd-tile iw, t-group tg.
                Split into 2-matmul units + a trailing bias/drain unit (the
                drain lags its chain by one pump slot to avoid head-of-line
                blocking on the DVE queue)."""
                units = []
                for wall, dst, bcol in ((wkall, kt, 2 + iw), (wqall, qt, iw)):
                    box = {}

                    def mk_mm(c0, wall=wall, box=box):
                        def f():
                            if c0 == 0:
                                box['ps'] = mmp.tile([P, TG], F32, name="mm",
                                                     tag="mm")
                            for c in (c0, c0 + 1):
                                nc.tensor.matmul(
                                    box['ps'], lhsT=wsl(wall, c, iw),
                                    rhs=xsl(tg, c),
                                    start=(c == 0), stop=(c == CCH - 1))
                        return f

                    def mk_bias(dst=dst, bcol=bcol, box=box):
                        def f():
                            nc.vector.tensor_scalar_add(
                                dst[iw][:, tg * TG:(tg + 1) * TG], box['ps'],
                                bqk_t[:, bcol:bcol + 1])
                        return f

                    units += [mk_mm(0), mk_mm(2), mk_mm(4), mk_mm(6),
                              mk_bias()]
                return units

            def v_chain_units(j):
                box = {}

                def mk_mm(c0):
                    def f():
                        if c0 == 0:
                            box['ps'] = mmp.tile([P, TG], F32, name="mm",
                                                 tag="mm")
                        for c in range(c0, c0 + 4):
                            nc.tensor.matmul(
                                box['ps'][:, :DD], lhsT=xvsl(j, c),
                                rhs=wvall[:, c * DD:(c + 1) * DD],
                                start=(c == 0), stop=(c == CCH - 1))
                    return f

                def drain():
                    ps = box['ps']
                    vview = vaug[j].rearrange("p (h x) -> p h x", h=NH)
                    bvv = bv_sb.rearrange("p (h x) -> p h x", h=NH)
                    # ones column (softmax denominator row): in0*0 + 1
                    nc.vector.tensor_scalar(
                        vview[:, :, D:D + 1], bvv[:, :, 0:1], 0.0, 1.0,
                        mybir.AluOpType.mult, mybir.AluOpType.add)
                    nc.vector.tensor_add(
                        vview[:, :, 0:D],
                        ps[:, :DD].rearrange("p (h x) -> p h x", h=NH), bvv)

                return [mk_mm(0), mk_mm(4), drain]

            def proj_units(tt, ec):
                box = {}

                def mm():
                    box['ps'] = mmp.tile([P, TG], F32, name="mm", tag="mm")
                    for i2 in range(2):
                        nc.tensor.matmul(
                            box['ps'], lhsT=ot[i2][:, tt * P:(tt + 1) * P],
                            rhs=wp[i2][:, ec * TG:(ec + 1) * TG],
                            start=(i2 == 0), stop=(i2 == 1))

                def drain():
                    ysb = ysbp.tile([P, TG], BF16, name="ysb", tag="ysb")
                    nc.vector.tensor_copy(ysb, box['ps'])
                    nc.sync.dma_start(
                        out=yp[tt * P:(tt + 1) * P, ec * TG:(ec + 1) * TG],
                        in_=ysb)

                return [mm, drain]

            # ---- attention ----
            def attn_block(i, g, fq):
                # causal: only s-chunks on/below the diagonal contribute.
                nj = NT if general_mask else 4 * g + 4
                av = [avp.tile([P, TG], F32, name="av", tag="av")
                      for _ in range(2)]
                pump_n = 2 if nj >= 12 else 1

                def emit_S(j):
                    st = stp.tile([P, 2 * TG], F32, name="st", tag="st")
                    for h in range(2):
                        nc.tensor.matmul(
                            st[:, h * TG:(h + 1) * TG],
                            lhsT=(kt[i][64 * h:64 * h + 64,
                                        j * P:(j + 1) * P]),
                            rhs=(qt[i][64 * h:64 * h + 64,
                                       g * TG:(g + 1) * TG]),
                            start=True, stop=True,
                            tile_position=(64 * h, 0))
                    r = j - 4 * g
                    if general_mask:
                        mk = mkp.tile([P, TG], F32, name="mk", tag="mk")
                        nc.sync.dma_start(
                            out=mk,
                            in_=maskT[j * P:(j + 1) * P, g * TG:(g + 1) * TG])
                        for h in range(2):
                            nc.vector.tensor_add(
                                st[:, h * TG:(h + 1) * TG],
                                st[:, h * TG:(h + 1) * TG], mk)
                    elif r >= 0:
                        for h in range(2):
                            sl = slice(h * TG + r * P, h * TG + (r + 1) * P)
                            nc.vector.tensor_add(st[:, sl], st[:, sl], band_t)
                    if apply_kbias:
                        for h in range(2):
                            nc.vector.tensor_scalar_add(
                                st[:, h * TG:(h + 1) * TG],
                                st[:, h * TG:(h + 1) * TG],
                                kbias_t[:, j:j + 1])
                    pt = ptp.tile([P, 2 * TG], BF16, name="pt", tag="pt")
                    nc.scalar.activation(pt, st, EXP, scale=1.0 / math.sqrt(D))
                    return pt

                def emit_AV(j, pt):
                    r = j - 4 * g
                    trim = r * P if (r > 0 and not general_mask) else 0
                    for h in range(2):
                        nc.tensor.matmul(
                            av[h][0:D + 1, trim:TG],
                            lhsT=(vaug[j][:, (2 * i + h) * (D + 1):
                                          (2 * i + h + 1) * (D + 1)]),
                            rhs=(pt[:, h * TG + trim:(h + 1) * TG]),
                            start=(j == 0), stop=(j == nj - 1),
                            skip_group_check=True)

                prev = None
                for j in range(nj):
                    pt = emit_S(j)
                    if prev is not None:
                        emit_AV(*prev)
                    prev = (j, pt)
                    if j >= 1:
                        fq.pump(pump_n)
                emit_AV(*prev)

                # Epilogue: free the accumulator banks, reciprocate the
                # denominator row via the DRAM transpose bounce, normalize.
                slot = i * NG + g
                asb = asbp.tile([D + 1, 2 * TG], BF16, name="asb", tag="asb")
                nc.vector.tensor_copy(asb[:, 0:TG], av[0][0:D + 1, :])
                nc.vector.tensor_copy(asb[:, TG:2 * TG], av[1][0:D + 1, :])
                nc.gpsimd.dma_start(out=rcd_raw[slot], in_=asb[D:D + 1, :])
                rt = rtp.tile([P, 8], BF16, name="rt", tag="rt")
                nc.gpsimd.dma_start(out=rt, in_=bass.AP(
                    tensor=rcd_raw.tensor, offset=rcd_raw[slot].offset,
                    ap=[[8, P], [1, 8]]))
                rw = rtp.tile([P, 8], BF16, name="rw", tag="rw")
                with nc.allow_low_precision(
                        reason="bf16 softmax denominators (~0.4% rel)"):
                    nc.vector.reciprocal(rw, rt)
                nc.gpsimd.dma_start(out=bass.AP(
                    tensor=rcd_rcp.tensor, offset=rcd_rcp[slot].offset,
                    ap=[[8, P], [1, 8]]), in_=rw)
                bc = bcp.tile([P, 2 * TG], BF16, name="bc", tag="bc")
                nc.gpsimd.dma_start(out=bc[0:D, :], in_=bass.AP(
                    tensor=rcd_rcp.tensor, offset=rcd_rcp[slot].offset,
                    ap=[[0, D], [1, 2 * TG]]))
                nc.vector.tensor_mul(
                    ot[i][0:D, g * TG:(g + 1) * TG],
                    asb[0:D, 0:TG], bc[0:D, 0:TG])
                tm = tmpp.tile([P, TG], BF16, name="tm", tag="tm")
                nc.vector.tensor_mul(tm[0:D, :], asb[0:D, TG:2 * TG],
                                     bc[0:D, TG:2 * TG])
                nc.sync.dma_start(
                    out=ot[i][D:P, g * TG:(g + 1) * TG],
                    in_=tm[0:D, :])

            # ---- emission schedule ----
            # Pair 0's first chains + V(0..3) run before its g=0 block; the
            # rest of QKV, pair-1 chains and the output projections are fed
            # through the filler queue into the attention blocks' spare PE
            # slots (attention is exp-paced on ScalarE).
            fq = FillerQueue()
            fq.add(qk_chain_units(0, 0))
            for j in range(4):
                fq.add(v_chain_units(j))
            fq.flush()
            for g in range(NG):
                if g < NG - 1:
                    fq.add(qk_chain_units(0, g + 1))
                    for j in range(4 * (g + 1), 4 * (g + 2)):
                        fq.add(v_chain_units(j))
                else:
                    fq.add(qk_chain_units(1, 0))
                attn_block(0, g, fq)
                fq.flush()
            for g in range(NG):
                if g < NG - 1:
                    fq.add(qk_chain_units(1, g + 1))
                attn_block(1, g, fq)
                fq.flush()
                for tt in range(4 * g, 4 * g + 4):
                    for ec in range(2):
                        fq.add(proj_units(tt, ec))
            fq.flush()

    nc.compile()
    return nc


@lru_cache(maxsize=4)
def _program(apply_kbias: bool, general_mask: bool) -> bass.Bass:
    return build_program(apply_kbias, general_mask)


def _host_prep(inputs):
    x = np.asarray(inputs["x"], np.float32)
    Wq = np.asarray(inputs["Wq"], np.float32)
    bq = np.asarray(inputs["bq"], np.float32)
    Wk = np.asarray(inputs["Wk"], np.float32)
    bk = np.asarray(inputs["bk"], np.float32)
    Wv = np.asarray(inputs["Wv"], np.float32)
    bv = np.asarray(inputs["bv"], np.float32)
    Wp = np.asarray(inputs["Wp"], np.float32)
    attn_mask = np.asarray(inputs["attn_mask"])
    valid = np.asarray(inputs["valid_input_mask"])

    tril = np.tril(np.ones((T, T), attn_mask.dtype))
    causal = all(np.array_equal(attn_mask[b], tril) for b in range(B))
    kbias_all = (valid.astype(np.float32) - 1.0) * 1e6  # [B, T]
    apply_kbias = bool((valid == 0).any())

    band = np.where(np.arange(P)[:, None] <= np.arange(P)[None, :],
                    np.float32(0.0), np.float32(NEG))

    in_maps = []
    for core in range(NCORES):
        b, hg = divmod(core, 4)
        sl = slice(hg * DD, (hg + 1) * DD)
        m = {
            "xT": np.ascontiguousarray(x[b].T).astype(ml_dtypes.bfloat16),
            "wqT": np.ascontiguousarray(Wq[sl, :].T).astype(ml_dtypes.bfloat16),
            "wkT": np.ascontiguousarray(Wk[sl, :].T).astype(ml_dtypes.bfloat16),
            "wvT": np.ascontiguousarray(Wv[sl, :].T).astype(ml_dtypes.bfloat16),
            "wpT": np.ascontiguousarray(Wp[:, sl].T).astype(ml_dtypes.bfloat16),
            "bqk": np.ascontiguousarray(
                np.stack([bq[sl][:P], bq[sl][P:], bk[sl][:P], bk[sl][P:]], 1)),
            "bv_sb": np.ascontiguousarray(np.tile(bv[sl], (P, 1))),
        }
        if apply_kbias:
            m["kbias"] = np.ascontiguousarray(kbias_all[b].reshape(NT, P).T)
        if not causal:
            m["maskT"] = np.ascontiguousarray(
                (attn_mask[b].T.astype(np.float32) - 1.0) * (-NEG))
        else:
            m["band"] = band
        in_maps.append(m)
    return in_maps, apply_kbias, causal


def _run(inputs, trace=False, trace_cores=None):
    global LAST_RESULTS
    in_maps, apply_kbias, causal = _host_prep(inputs)
    nc = _program(apply_kbias, not causal)
    res = bass_utils.run_bass_kernel_spmd(
        nc, in_maps, core_ids=list(range(NCORES)), trace=trace,
        trace_cores=trace_cores)
    LAST_RESULTS = res

    bp = np.asarray(inputs["bp"], np.float32)
    y = np.zeros((B, T, C), np.float32)
    for core in range(NCORES):
        y[core // 4] += np.asarray(res.results[core]["yp"], np.float32)
    y += bp[None, None, :]
    return y


def kernel(**inputs) -> np.ndarray:
    return _run(inputs)


# revision 9
# speedup vs baseline: 1.0660x; 1.0026x over previous
"""Causal self-attention (B=2, T=2048, C=1024, H=16) on 8 TRN2 NeuronCores.

Sharding: core = (batch b, head-group hg) with b in {0,1}, hg in {0..3};
each core computes Q/K/V projections and attention for its 4 heads on its
batch, plus the row-parallel slice of the output projection. The host sums
the 4 per-core partial projections per batch (bf16) and adds the output bias.

Device algorithm (all layouts transposed so softmax needs no on-chip
transposes):
  - x streamed t-group-major ([c, tg] tiles via one 3D-AP DMA per tg) so the
    K/Q projection chains start ~4us after launch; a burst of tiny warm-up
    matmuls releases the HAM clock throttle before the first real chain.
  - Q^T, K^T [dd, t] and V [t, dd] via bf16 matmul chains (contraction over C).
  - S^T[s, t]: 2 heads row-packed in the PE array (K=64 at row offsets 0/64).
  - exp on ScalarE straight out of PSUM (scale=1/sqrt(d) folded in); causal
    masking = one additive 128x128 band on diagonal blocks + trimming the
    AV matmul's moving range; softmax denominators from an all-ones column
    appended to V (M=65 matmul); normalization deferred to after AV.
  - softmax reciprocals: denominator row bounced through DRAM into a
    [128, 8] partition-major tile, one DVE reciprocal op (ScalarE runs
    exps only), bounced back and broadcast-read for the normalize muls.
  - Emission interleaves projection/QKV chains into the attention blocks'
    spare PE slots (attention is ScalarE-exp paced) so the PE never idles.
  - y_partial[t, e] bf16 out; host sums partials in f32.
"""

import math
from collections import deque
from functools import lru_cache

import ml_dtypes
import numpy as np

import concourse.bass as bass
import concourse.mybir as mybir
from concourse import bacc
import concourse.tile as tile
from concourse import bass_utils

F32 = mybir.dt.float32
BF16 = mybir.dt.bfloat16
EXP = mybir.ActivationFunctionType.Exp

B, T, C, H = 2, 2048, 1024, 16
NCORES = 8
NH = 4            # heads per core
D = C // H        # 64
DD = NH * D       # 256 channels per core
P = 128
TG = 512          # t-group width (matmul moving dim)
NG = T // TG      # 4
NT = T // P       # 16 s-chunks
CCH = C // P      # 8 contraction chunks
NEG = -8.0e6      # pre-scale additive mask; *0.125 = -1e6 like the reference

LAST_RESULTS = None  # BassKernelResults of the most recent run (for test.py)


class FillerQueue:
    """PE work units interleaved into the attention blocks' spare slots."""

    def __init__(self):
        self.q = deque()

    def add(self, units):
        self.q.extend(units)

    def pump(self, n=1):
        for _ in range(n):
            if not self.q:
                return
            self.q.popleft()()

    def flush(self):
        while self.q:
            self.q.popleft()()


def build_program(apply_kbias: bool, general_mask: bool) -> bass.Bass:
    nc = bacc.Bacc("TRN2", target_bir_lowering=False, debug=False,
                   enable_asserts=False)

    xT = nc.dram_tensor("xT", [C, T], BF16, kind="ExternalInput").ap()
    wqT = nc.dram_tensor("wqT", [C, DD], BF16, kind="ExternalInput").ap()
    wkT = nc.dram_tensor("wkT", [C, DD], BF16, kind="ExternalInput").ap()
    wvT = nc.dram_tensor("wvT", [C, DD], BF16, kind="ExternalInput").ap()
    wpT = nc.dram_tensor("wpT", [DD, C], BF16, kind="ExternalInput").ap()
    bqk = nc.dram_tensor("bqk", [P, 4], F32, kind="ExternalInput").ap()
    bv_in = nc.dram_tensor("bv_sb", [P, DD], F32, kind="ExternalInput").ap()
    kbias_in = None
    if apply_kbias:
        kbias_in = nc.dram_tensor("kbias", [P, NT], F32, kind="ExternalInput").ap()
    band_in = maskT = None
    if general_mask:
        maskT = nc.dram_tensor("maskT", [T, T], F32, kind="ExternalInput").ap()
    else:
        band_in = nc.dram_tensor("band", [P, P], F32, kind="ExternalInput").ap()
    yp = nc.dram_tensor("yp", [T, C], BF16, kind="ExternalOutput").ap()
    # DRAM bounce buffers for the softmax denominators: raw rows land in
    # rcd_raw, get re-read [128, 8] partition-major (contiguous 8-elem lines),
    # reciprocated on DVE, written back t-major to rcd_rcp, then broadcast
    # across partitions (DMA from DRAM may use a 0-step partition dim).
    rcd_raw = nc.dram_tensor("rcd_raw", [2 * NG, 2 * TG], BF16, kind="Internal").ap()
    rcd_rcp = nc.dram_tensor("rcd_rcp", [2 * NG, 2 * TG], BF16, kind="Internal").ap()

    with tile.TileContext(nc) as tc:
        with tc.tile_pool(name="wts", bufs=1) as wts, \
             tc.tile_pool(name="xtp", bufs=1) as xtp, \
             tc.tile_pool(name="qkv", bufs=1) as qkv, \
             tc.tile_pool(name="otp", bufs=1) as otp, \
             tc.tile_pool(name="ptp", bufs=4) as ptp, \
             tc.tile_pool(name="asb", bufs=4) as asbp, \
             tc.tile_pool(name="rtp", bufs=2) as rtp, \
             tc.tile_pool(name="bcp", bufs=2) as bcp, \
             tc.tile_pool(name="tmp", bufs=2) as tmpp, \
             tc.tile_pool(name="ysb", bufs=2) as ysbp, \
             tc.tile_pool(name="mkp", bufs=2) as mkp, \
             tc.tile_pool(name="stp", bufs=2, space="PSUM") as stp, \
             tc.tile_pool(name="avp", bufs=2, space="PSUM") as avp, \
             tc.tile_pool(name="mmp", bufs=2, space="PSUM") as mmp:

            # Only Exp (and friends) are needed; preload so the act-table
            # DMA overlaps the input DMAs instead of stalling the first exp.
            from concourse.hw_specs import get_activation_tables
            tables = get_activation_tables(nc.m.arch)
            set_id = list(tables).index("exp_and_others")
            nc.scalar.add_instruction(mybir.InstLoadActFuncSet(
                name=nc.get_next_instruction_name(), ins=[], outs=[],
                act_func_set_id=set_id))

            # ---- input DMAs ----
            bqk_t = wts.tile([P, 4], F32, name="bqk_t")
            nc.sync.dma_start(out=bqk_t, in_=bqk)

            # weights gathered c-major into single wide tiles (1 DMA each):
            # w[p, c*DD+q] = wT[c*P+p, q]
            def w_src(wT):
                return bass.AP(tensor=wT.tensor, offset=wT.offset,
                               ap=[[DD, P], [P * DD, CCH], [1, DD]])

            wkall = wts.tile([P, CCH * DD], BF16, name="wkall")
            wqall = wts.tile([P, CCH * DD], BF16, name="wqall")
            wvall = wts.tile([P, CCH * DD], BF16, name="wvall")
            # x gathered t-group-major: xtg[tg][p, c*TG+u] = xT[c*P+p, tg*TG+u]
            xtg = [xtp.tile([P, CCH * TG], BF16, name=f"xtg{t_}")
                   for t_ in range(NG)]

            def x_src(tg):
                return bass.AP(tensor=xT.tensor, offset=tg * TG,
                               ap=[[T, P], [P * T, CCH], [1, TG]])

            # x tiles stream on the gpsimd DMA queue concurrently with the
            # weights on the sync queue so the first chains start ~5us in.
            nc.sync.dma_start(out=wkall, in_=w_src(wkT))
            nc.gpsimd.dma_start(out=xtg[0], in_=x_src(0))
            nc.sync.dma_start(out=wqall, in_=w_src(wqT))
            nc.sync.dma_start(out=wvall, in_=w_src(wvT))
            for t_ in range(1, NG):
                nc.gpsimd.dma_start(out=xtg[t_], in_=x_src(t_))
            bv_sb = wts.tile([P, DD], F32, name="bv_t")
            nc.sync.dma_start(out=bv_sb, in_=bv_in)
            if band_in is not None:
                band_t = wts.tile([P, P], F32, name="band_t")
                nc.sync.dma_start(out=band_t, in_=band_in)
            if kbias_in is not None:
                kbias_t = wts.tile([P, NT], F32, name="kbias_t")
                nc.sync.dma_start(out=kbias_t, in_=kbias_in)
            wp = [wts.tile([P, C], BF16, name=f"wp{i}") for i in range(2)]
            for i in range(2):
                nc.sync.dma_start(out=wp[i], in_=wpT[i * P:(i + 1) * P, :])

            qt = [qkv.tile([P, T], BF16, name=f"qt{i}") for i in range(2)]
            kt = [qkv.tile([P, T], BF16, name=f"kt{i}") for i in range(2)]
            vaug = [qkv.tile([P, NH * (D + 1)], BF16, name=f"vaug{j}")
                    for j in range(NT)]
            ot = [otp.tile([P, T], BF16, name=f"ot{i}") for i in range(2)]

            # ---- HAM warm-up: ~3us of tiny matmuls so the PE clock is at
            # 8/8 by the time the first projection chain lands.
            wps = mmp.tile([P, TG], F32, name="mm", tag="mm")
            for _ in range(64):
                nc.tensor.matmul(wps[0:1, 0:1], lhsT=bqk_t[:, 0:1],
                                 rhs=bqk_t[:, 0:1], start=True, stop=True)

            # ---- QKV chain units ----
            def wsl(wall, c, iw):
                return wall[:, c * DD + iw * P: c * DD + iw * P + P]

            def xsl(tg, c):
                return xtg[tg][:, c * TG:(c + 1) * TG]

            def xvsl(j, c):
                tg, u = divmod(j, NG)
                return xtg[tg][:, c * TG + u * P: c * TG + u * P + P]

            def qk_chain_units(iw, tg):
                """K then Q projection chain for dd-tile iw, t-group tg.
                Split into 2-matmul units + a trailing bias/drain unit (the
                drain lags its chain by one pump slot to avoid head-of-line
                blocking on the DVE queue)."""
                units = []
                for wall, dst, bcol in ((wkall, kt, 2 + iw), (wqall, qt, iw)):
                    box = {}

                    def mk_mm(c0, wall=wall, box=box):
                        def f():
                            if c0 == 0:
                                box['ps'] = mmp.tile([P, TG], F32, name="mm",
                                                     tag="mm")
                            for c in (c0, c0 + 1):
                                nc.tensor.matmul(
                                    box['ps'], lhsT=wsl(wall, c, iw),
                                    rhs=xsl(tg, c),
                                    start=(c == 0), stop=(c == CCH - 1))
                        return f

                    def mk_bias(dst=dst, bcol=bcol, box=box):
                        def f():
                            nc.vector.tensor_scalar_add(
                                dst[iw][:, tg * TG:(tg + 1) * TG], box['ps'],
                                bqk_t[:, bcol:bcol + 1])
                        return f

                    units += [mk_mm(0), mk_mm(2), mk_mm(4), mk_mm(6),
                              mk_bias()]
                return units

            def v_chain_units(j):
                box = {}

                def mk_mm(c0):
                    def f():
                        if c0 == 0:
                            box['ps'] = mmp.tile([P, TG], F32, name="mm",
                                                 tag="mm")
                        for c in range(c0, c0 + 4):
                            nc.tensor.matmul(
                                box['ps'][:, :DD], lhsT=xvsl(j, c),
                                rhs=wvall[:, c * DD:(c + 1) * DD],
                                start=(c == 0), stop=(c == CCH - 1))
                    return f

                def drain():
                    ps = box['ps']
                    vview = vaug[j].rearrange("p (h x) -> p h x", h=NH)
                    bvv = bv_sb.rearrange("p (h x) -> p h x", h=NH)
                    # ones column (softmax denominator row): in0*0 + 1
                    nc.vector.tensor_scalar(
                        vview[:, :, D:D + 1], bvv[:, :, 0:1], 0.0, 1.0,
                        mybir.AluOpType.mult, mybir.AluOpType.add)
                    nc.vector.tensor_add(
                        vview[:, :, 0:D],
                        ps[:, :DD].rearrange("p (h x) -> p h x", h=NH), bvv)

                return [mk_mm(0), mk_mm(4), drain]

            def proj_units(tt, ec):
                box = {}
                alt = (tt * 2 + ec) % 2

                def mm():
                    box['ps'] = mmp.tile([P, TG], F32, name="mm", tag="mm")
                    for i2 in range(2):
                        nc.tensor.matmul(
                            box['ps'], lhsT=ot[i2][:, tt * P:(tt + 1) * P],
                            rhs=wp[i2][:, ec * TG:(ec + 1) * TG],
                            start=(i2 == 0), stop=(i2 == 1))

                def drain():
                    # alternate the PSUM->SBUF cast and the store DMA across
                    # engines/queues so the drains pipeline 2-wide
                    ysb = ysbp.tile([P, TG], BF16, name="ysb", tag="ysb")
                    if alt:
                        nc.scalar.activation(
                            ysb, box['ps'], mybir.ActivationFunctionType.Copy)
                        nc.gpsimd.dma_start(
                            out=yp[tt * P:(tt + 1) * P,
                                   ec * TG:(ec + 1) * TG], in_=ysb)
                    else:
                        nc.vector.tensor_copy(ysb, box['ps'])
                        nc.sync.dma_start(
                            out=yp[tt * P:(tt + 1) * P,
                                   ec * TG:(ec + 1) * TG], in_=ysb)

                return [mm, drain]

            # ---- attention ----
            def attn_block(i, g, fq, pending=None, defer_av=False):
                # causal: only s-chunks on/below the diagonal contribute.
                # `pending` is the previous block's deferred epilogue tail,
                # emitted after S(1) so its DMA-bounce waits never block this
                # block's DVE stream. `defer_av` emits all S's before any AV
                # (first block: the V chains feeding AV are still in fq).
                nj = NT if general_mask else 4 * g + 4
                av = [avp.tile([P, TG], F32, name="av", tag="av")
                      for _ in range(2)]
                pump_n = 2 if nj >= 12 else 1

                def emit_S(j):
                    st = stp.tile([P, 2 * TG], F32, name="st", tag="st")
                    for h in range(2):
                        nc.tensor.matmul(
                            st[:, h * TG:(h + 1) * TG],
                            lhsT=(kt[i][64 * h:64 * h + 64,
                                        j * P:(j + 1) * P]),
                            rhs=(qt[i][64 * h:64 * h + 64,
                                       g * TG:(g + 1) * TG]),
                            start=True, stop=True,
                            tile_position=(64 * h, 0))
                    r = j - 4 * g
                    if general_mask:
                        mk = mkp.tile([P, TG], F32, name="mk", tag="mk")
                        nc.sync.dma_start(
                            out=mk,
                            in_=maskT[j * P:(j + 1) * P, g * TG:(g + 1) * TG])
                        for h in range(2):
                            nc.vector.tensor_add(
                                st[:, h * TG:(h + 1) * TG],
                                st[:, h * TG:(h + 1) * TG], mk)
                    elif r >= 0:
                        for h in range(2):
                            sl = slice(h * TG + r * P, h * TG + (r + 1) * P)
                            nc.vector.tensor_add(st[:, sl], st[:, sl], band_t)
                    if apply_kbias:
                        for h in range(2):
                            nc.vector.tensor_scalar_add(
                                st[:, h * TG:(h + 1) * TG],
                                st[:, h * TG:(h + 1) * TG],
                                kbias_t[:, j:j + 1])
                    pt = ptp.tile([P, 2 * TG], BF16, name="pt", tag="pt")
                    nc.scalar.activation(pt, st, EXP, scale=1.0 / math.sqrt(D))
                    return pt

                def emit_AV(j, pt):
                    r = j - 4 * g
                    trim = r * P if (r > 0 and not general_mask) else 0
                    for h in range(2):
                        nc.tensor.matmul(
                            av[h][0:D + 1, trim:TG],
                            lhsT=(vaug[j][:, (2 * i + h) * (D + 1):
                                          (2 * i + h + 1) * (D + 1)]),
                            rhs=(pt[:, h * TG + trim:(h + 1) * TG]),
                            start=(j == 0), stop=(j == nj - 1),
                            skip_group_check=True)

                prev = None
                for j in range(nj):
                    pt = emit_S(j)
                    if j == 1 and pending is not None:
                        pending()
                        pending = None
                    if prev is not None and not defer_av:
                        emit_AV(*prev)
                    if not defer_av:
                        prev = (j, pt)
                    else:
                        prev = prev or []
                        prev.append((j, pt))
                    if j >= 1:
                        fq.pump(pump_n)
                if defer_av:
                    fq.flush()
                    for j, pt in prev:
                        emit_AV(j, pt)
                else:
                    emit_AV(*prev)

                # Epilogue part A: free the accumulator banks, launch the
                # denominator row into the DRAM transpose bounce.
                slot = i * NG + g
                asb = asbp.tile([D + 1, 2 * TG], BF16, name="asb", tag="asb")
                nc.vector.tensor_copy(asb[:, 0:TG], av[0][0:D + 1, :])
                nc.vector.tensor_copy(asb[:, TG:2 * TG], av[1][0:D + 1, :])
                nc.gpsimd.dma_start(out=rcd_raw[slot], in_=asb[D:D + 1, :])
                rt = rtp.tile([P, 8], BF16, name="rt", tag="rt")
                nc.gpsimd.dma_start(out=rt, in_=bass.AP(
                    tensor=rcd_raw.tensor, offset=rcd_raw[slot].offset,
                    ap=[[8, P], [1, 8]]))

                # Epilogue part B (deferred into the next block so the
                # bounce round-trips never stall this DVE/sync stream).
                def part_b():
                    rw = rtp.tile([P, 8], BF16, name="rw", tag="rw")
                    with nc.allow_low_precision(
                            reason="bf16 softmax denominators (~0.4% rel)"):
                        nc.vector.reciprocal(rw, rt)
                    nc.gpsimd.dma_start(out=bass.AP(
                        tensor=rcd_rcp.tensor, offset=rcd_rcp[slot].offset,
                        ap=[[8, P], [1, 8]]), in_=rw)
                    bc = bcp.tile([P, 2 * TG], BF16, name="bc", tag="bc")
                    nc.gpsimd.dma_start(out=bc[0:D, :], in_=bass.AP(
                        tensor=rcd_rcp.tensor, offset=rcd_rcp[slot].offset,
                        ap=[[0, D], [1, 2 * TG]]))
                    nc.vector.tensor_mul(
                        ot[i][0:D, g * TG:(g + 1) * TG],
                        asb[0:D, 0:TG], bc[0:D, 0:TG])
                    tm = tmpp.tile([P, TG], BF16, name="tm", tag="tm")
                    nc.vector.tensor_mul(tm[0:D, :], asb[0:D, TG:2 * TG],
                                         bc[0:D, TG:2 * TG])
                    nc.sync.dma_start(
                        out=ot[i][D:P, g * TG:(g + 1) * TG],
                        in_=tm[0:D, :])

                return part_b

            # ---- emission schedule ----
            # Pair 0's first chains + V(0..3) run before its g=0 block; the
            # rest of QKV, pair-1 chains and the output projections are fed
            # through the filler queue into the attention blocks' spare PE
            # slots (attention is exp-paced on ScalarE).
            fq = FillerQueue()
            fq.add(qk_chain_units(0, 0))
            fq.flush()
            for j in range(4):
                fq.add(v_chain_units(j))
            pend = None
            for g in range(NG):
                if g < NG - 1:
                    fq.add(qk_chain_units(0, g + 1))
                    for j in range(4 * (g + 1), 4 * (g + 2)):
                        fq.add(v_chain_units(j))
                else:
                    fq.add(qk_chain_units(1, 0))
                pend = attn_block(0, g, fq, pending=pend, defer_av=(g == 0))
                fq.flush()
            for g in range(NG):
                if g < NG - 1:
                    fq.add(qk_chain_units(1, g + 1))
                pend = attn_block(1, g, fq, pending=pend)
                fq.flush()
                for tt in range(4 * g, 4 * g + 4):
                    for ec in range(2):
                        fq.add(proj_units(tt, ec))
            pend()
            fq.flush()

    nc.compile()
    return nc


@lru_cache(maxsize=4)
def _program(apply_kbias: bool, general_mask: bool) -> bass.Bass:
    return build_program(apply_kbias, general_mask)


def _host_prep(inputs):
    x = np.asarray(inputs["x"], np.float32)
    Wq = np.asarray(inputs["Wq"], np.float32)
    bq = np.asarray(inputs["bq"], np.float32)
    Wk = np.asarray(inputs["Wk"], np.float32)
    bk = np.asarray(inputs["bk"], np.float32)
    Wv = np.asarray(inputs["Wv"], np.float32)
    bv = np.asarray(inputs["bv"], np.float32)
    Wp = np.asarray(inputs["Wp"], np.float32)
    attn_mask = np.asarray(inputs["attn_mask"])
    valid = np.asarray(inputs["valid_input_mask"])

    tril = np.tril(np.ones((T, T), attn_mask.dtype))
    causal = all(np.array_equal(attn_mask[b], tril) for b in range(B))
    kbias_all = (valid.astype(np.float32) - 1.0) * 1e6  # [B, T]
    apply_kbias = bool((valid == 0).any())

    band = np.where(np.arange(P)[:, None] <= np.arange(P)[None, :],
                    np.float32(0.0), np.float32(NEG))

    in_maps = []
    for core in range(NCORES):
        b, hg = divmod(core, 4)
        sl = slice(hg * DD, (hg + 1) * DD)
        m = {
            "xT": np.ascontiguousarray(x[b].T).astype(ml_dtypes.bfloat16),
            "wqT": np.ascontiguousarray(Wq[sl, :].T).astype(ml_dtypes.bfloat16),
            "wkT": np.ascontiguousarray(Wk[sl, :].T).astype(ml_dtypes.bfloat16),
            "wvT": np.ascontiguousarray(Wv[sl, :].T).astype(ml_dtypes.bfloat16),
            "wpT": np.ascontiguousarray(Wp[:, sl].T).astype(ml_dtypes.bfloat16),
            "bqk": np.ascontiguousarray(
                np.stack([bq[sl][:P], bq[sl][P:], bk[sl][:P], bk[sl][P:]], 1)),
            "bv_sb": np.ascontiguousarray(np.tile(bv[sl], (P, 1))),
        }
        if apply_kbias:
            m["kbias"] = np.ascontiguousarray(kbias_all[b].reshape(NT, P).T)
        if not causal:
            m["maskT"] = np.ascontiguousarray(
                (attn_mask[b].T.astype(np.float32) - 1.0) * (-NEG))
        else:
            m["band"] = band
        in_maps.append(m)
    return in_maps, apply_kbias, causal


def _run(inputs, trace=False, trace_cores=None):
    global LAST_RESULTS
    in_maps, apply_kbias, causal = _host_prep(inputs)
    nc = _program(apply_kbias, not causal)
    res = bass_utils.run_bass_kernel_spmd(
        nc, in_maps, core_ids=list(range(NCORES)), trace=trace,
        trace_cores=trace_cores)
    LAST_RESULTS = res

    bp = np.asarray(inputs["bp"], np.float32)
    y = np.zeros((B, T, C), np.float32)
    for core in range(NCORES):
        y[core // 4] += np.asarray(res.results[core]["yp"], np.float32)
    y += bp[None, None, :]
    return y


def kernel(**inputs) -> np.ndarray:
    return _run(inputs)


# revision 14
# speedup vs baseline: 1.1721x; 1.0995x over previous
"""Causal self-attention (B=2, T=2048, C=1024, H=16) on 8 TRN2 NeuronCores.

Sharding: core = (batch b, head-group hg) with b in {0,1}, hg in {0..3};
each core computes Q/K/V projections and attention for its 4 heads on its
batch, plus the row-parallel slice of the output projection. The host sums
the 4 per-core partial projections per batch (bf16) and adds the output bias.

Device algorithm (all layouts transposed so softmax needs no on-chip
transposes):
  - x streamed t-group-major ([c, tg] tiles via one 3D-AP DMA per tg) so the
    K/Q projection chains start ~4us after launch; a burst of tiny warm-up
    matmuls releases the HAM clock throttle before the first real chain.
  - Q^T, K^T [dd, t] and V [t, dd] via bf16 matmul chains (contraction over C).
  - S^T[s, t]: 2 heads row-packed in the PE array (K=64 at row offsets 0/64).
  - exp on ScalarE straight out of PSUM (scale=1/sqrt(d) folded in); causal
    masking = one additive 128x128 band on diagonal blocks + trimming the
    AV matmul's moving range; softmax denominators from an all-ones column
    appended to V (M=65 matmul); normalization deferred to after AV.
  - softmax reciprocals: denominator row bounced through DRAM into a
    [128, 8] partition-major tile, one DVE reciprocal op (ScalarE runs
    exps only), bounced back and broadcast-read for the normalize muls.
  - Emission interleaves projection/QKV chains into the attention blocks'
    spare PE slots (attention is ScalarE-exp paced) so the PE never idles.
  - y_partial[t, e] bf16 out; host sums partials in f32.
"""

import math
from collections import deque
from functools import lru_cache

import ml_dtypes
import numpy as np

import concourse.bass as bass
import concourse.mybir as mybir
from concourse import bacc
import concourse.tile as tile
from concourse import bass_utils

F32 = mybir.dt.float32
BF16 = mybir.dt.bfloat16
EXP = mybir.ActivationFunctionType.Exp

B, T, C, H = 2, 2048, 1024, 16
NCORES = 8
NH = 4            # heads per core
D = C // H        # 64
DD = NH * D       # 256 channels per core
P = 128
TG = 512          # t-group width (matmul moving dim)
NG = T // TG      # 4
NT = T // P       # 16 s-chunks
CCH = C // P      # 8 contraction chunks
NEG = -8.0e6      # pre-scale additive mask; *0.125 = -1e6 like the reference

LAST_RESULTS = None  # BassKernelResults of the most recent run (for test.py)


class FillerQueue:
    """PE work units interleaved into the attention blocks' spare slots."""

    def __init__(self):
        self.q = deque()

    def add(self, units):
        self.q.extend(units)

    def pump(self, n=1):
        for _ in range(n):
            if not self.q:
                return
            self.q.popleft()()

    def flush(self):
        while self.q:
            self.q.popleft()()


def build_program(apply_kbias: bool, general_mask: bool) -> bass.Bass:
    nc = bacc.Bacc("TRN2", target_bir_lowering=False, debug=False,
                   enable_asserts=False)

    xT = nc.dram_tensor("xT", [C, T], BF16, kind="ExternalInput").ap()
    wqT = nc.dram_tensor("wqT", [C, DD], BF16, kind="ExternalInput").ap()
    wkT = nc.dram_tensor("wkT", [C, DD], BF16, kind="ExternalInput").ap()
    wvT = nc.dram_tensor("wvT", [C, DD], BF16, kind="ExternalInput").ap()
    wpT = nc.dram_tensor("wpT", [DD, C], BF16, kind="ExternalInput").ap()
    bqk = nc.dram_tensor("bqk", [P, 4], F32, kind="ExternalInput").ap()
    bv_in = nc.dram_tensor("bv_sb", [P, DD], F32, kind="ExternalInput").ap()
    kbias_in = None
    if apply_kbias:
        kbias_in = nc.dram_tensor("kbias", [P, NT], F32, kind="ExternalInput").ap()
    band_in = maskT = None
    if general_mask:
        maskT = nc.dram_tensor("maskT", [T, T], F32, kind="ExternalInput").ap()
    else:
        band_in = nc.dram_tensor("band", [P, P], F32, kind="ExternalInput").ap()
    yp = nc.dram_tensor("yp", [T, C], BF16, kind="ExternalOutput").ap()
    # DRAM bounce buffers for the softmax denominators: raw rows land in
    # rcd_raw, get re-read [128, 8] partition-major (contiguous 8-elem lines),
    # reciprocated on DVE, written back t-major to rcd_rcp, then broadcast
    # across partitions (DMA from DRAM may use a 0-step partition dim).
    rcd_raw = nc.dram_tensor("rcd_raw", [2 * NG, 2 * TG], BF16, kind="Internal").ap()
    rcd_rcp = nc.dram_tensor("rcd_rcp", [2 * NG, 2 * TG], BF16, kind="Internal").ap()

    with tile.TileContext(nc) as tc:
        with tc.tile_pool(name="wts", bufs=1) as wts, \
             tc.tile_pool(name="xtp", bufs=1) as xtp, \
             tc.tile_pool(name="qkv", bufs=1) as qkv, \
             tc.tile_pool(name="otp", bufs=1) as otp, \
             tc.tile_pool(name="ptp", bufs=4) as ptp, \
             tc.tile_pool(name="asb", bufs=4) as asbp, \
             tc.tile_pool(name="rtp", bufs=2) as rtp, \
             tc.tile_pool(name="bcp", bufs=2) as bcp, \
             tc.tile_pool(name="tmp", bufs=3) as tmpp, \
             tc.tile_pool(name="ysb", bufs=6) as ysbp, \
             tc.tile_pool(name="mkp", bufs=2) as mkp, \
             tc.tile_pool(name="stp", bufs=2, space="PSUM") as stp, \
             tc.tile_pool(name="avp", bufs=2, space="PSUM") as avp, \
             tc.tile_pool(name="mmp", bufs=2, space="PSUM") as mmp:

            # Only Exp (and friends) are needed; preload so the act-table
            # DMA overlaps the input DMAs instead of stalling the first exp.
            from concourse.hw_specs import get_activation_tables
            tables = get_activation_tables(nc.m.arch)
            set_id = list(tables).index("natural_log_exp_and_others")
            nc.scalar.add_instruction(mybir.InstLoadActFuncSet(
                name=nc.get_next_instruction_name(), ins=[], outs=[],
                act_func_set_id=set_id))

            # ---- input DMAs ----
            bqk_t = wts.tile([P, 4], F32, name="bqk_t")
            nc.sync.dma_start(out=bqk_t, in_=bqk)

            # weights gathered c-major into single wide tiles (1 DMA each):
            # w[p, c*DD+q] = wT[c*P+p, q]
            def w_src(wT):
                return bass.AP(tensor=wT.tensor, offset=wT.offset,
                               ap=[[DD, P], [P * DD, CCH], [1, DD]])

            wkall = wts.tile([P, CCH * DD], BF16, name="wkall")
            wqall = wts.tile([P, CCH * DD], BF16, name="wqall")
            wvall = wts.tile([P, CCH * DD], BF16, name="wvall")
            # x gathered t-group-major: xtg[tg][p, c*TG+u] = xT[c*P+p, tg*TG+u]
            xtg = [xtp.tile([P, CCH * TG], BF16, name=f"xtg{t_}")
                   for t_ in range(NG)]

            def x_src(tg):
                return bass.AP(tensor=xT.tensor, offset=tg * TG,
                               ap=[[T, P], [P * T, CCH], [1, TG]])

            # xtg[0] streams on the gpsimd queue concurrently with the
            # weights on the sync queue; xtg[1..3] (3 MB, not needed until
            # ~t=16us) go at the END of the sync queue so they don't steal
            # DMA bandwidth from the first chains' weights.
            nc.sync.dma_start(out=wkall, in_=w_src(wkT))
            nc.gpsimd.dma_start(out=xtg[0], in_=x_src(0))
            nc.sync.dma_start(out=wqall, in_=w_src(wqT))
            nc.sync.dma_start(out=wvall, in_=w_src(wvT))
            bv_sb = wts.tile([P, DD], F32, name="bv_t")
            nc.sync.dma_start(out=bv_sb, in_=bv_in)
            if band_in is not None:
                band_t = wts.tile([P, P], F32, name="band_t")
                nc.sync.dma_start(out=band_t, in_=band_in)
            if kbias_in is not None:
                kbias_t = wts.tile([P, NT], F32, name="kbias_t")
                nc.sync.dma_start(out=kbias_t, in_=kbias_in)
            wp = [wts.tile([P, C], BF16, name=f"wp{i}") for i in range(2)]
            for i in range(2):
                nc.sync.dma_start(out=wp[i], in_=wpT[i * P:(i + 1) * P, :])
            for t_ in range(1, NG):
                nc.sync.dma_start(out=xtg[t_], in_=x_src(t_))

            qt = [qkv.tile([P, T], BF16, name=f"qt{i}") for i in range(2)]
            kt = [qkv.tile([P, T], BF16, name=f"kt{i}") for i in range(2)]
            vaug = [qkv.tile([P, NH * (D + 1)], BF16, name=f"vaug{j}")
                    for j in range(NT)]
            ot = [otp.tile([P, T], BF16, name=f"ot{i}") for i in range(2)]

            # ---- HAM warm-up: ~3us of tiny matmuls so the PE clock is at
            # 8/8 by the time the first projection chain lands.
            wps = mmp.tile([P, TG], F32, name="mm", tag="mm")
            for _ in range(100):
                nc.tensor.matmul(wps[0:1, 0:1], lhsT=bqk_t[:, 0:1],
                                 rhs=bqk_t[:, 0:1], start=True, stop=True)

            # ---- QKV chain units ----
            def wsl(wall, c, iw):
                return wall[:, c * DD + iw * P: c * DD + iw * P + P]

            def xsl(tg, c):
                return xtg[tg][:, c * TG:(c + 1) * TG]

            def xvsl(j, c):
                tg, u = divmod(j, NG)
                return xtg[tg][:, c * TG + u * P: c * TG + u * P + P]

            def qk_chain_units(iw, tg):
                """K then Q projection chain for dd-tile iw, t-group tg.
                Split into 2-matmul units + a trailing bias/drain unit (the
                drain lags its chain by one pump slot to avoid head-of-line
                blocking on the DVE queue)."""
                units = []
                for wall, dst, bcol in ((wkall, kt, 2 + iw), (wqall, qt, iw)):
                    box = {}

                    def mk_mm(c0, wall=wall, box=box):
                        def f():
                            if c0 == 0:
                                box['ps'] = mmp.tile([P, TG], F32, name="mm",
                                                     tag="mm")
                            for c in (c0, c0 + 1):
                                nc.tensor.matmul(
                                    box['ps'], lhsT=wsl(wall, c, iw),
                                    rhs=xsl(tg, c),
                                    start=(c == 0), stop=(c == CCH - 1))
                        return f

                    def mk_bias(dst=dst, bcol=bcol, box=box):
                        def f():
                            nc.vector.tensor_scalar_add(
                                dst[iw][:, tg * TG:(tg + 1) * TG], box['ps'],
                                bqk_t[:, bcol:bcol + 1])
                        return f

                    units += [mk_mm(0), mk_mm(2), mk_mm(4), mk_mm(6),
                              mk_bias()]
                return units

            def v_chain_units(j):
                box = {}

                def mk_mm(c0):
                    def f():
                        if c0 == 0:
                            box['ps'] = mmp.tile([P, TG], F32, name="mm",
                                                 tag="mm")
                        for c in range(c0, c0 + 4):
                            nc.tensor.matmul(
                                box['ps'][:, :DD], lhsT=xvsl(j, c),
                                rhs=wvall[:, c * DD:(c + 1) * DD],
                                start=(c == 0), stop=(c == CCH - 1))
                    return f

                def drain():
                    ps = box['ps']
                    vview = vaug[j].rearrange("p (h x) -> p h x", h=NH)
                    bvv = bv_sb.rearrange("p (h x) -> p h x", h=NH)
                    # ones column (softmax denominator row): in0*0 + 1
                    nc.vector.tensor_scalar(
                        vview[:, :, D:D + 1], bvv[:, :, 0:1], 0.0, 1.0,
                        mybir.AluOpType.mult, mybir.AluOpType.add)
                    nc.vector.tensor_add(
                        vview[:, :, 0:D],
                        ps[:, :DD].rearrange("p (h x) -> p h x", h=NH), bvv)

                return [mk_mm(0), mk_mm(4), drain]

            def proj_units(tt, ec):
                box = {}
                alt = (tt * 2 + ec) % 2

                def mm():
                    box['ps'] = mmp.tile([P, TG], F32, name="mm", tag="mm")
                    for i2 in range(2):
                        nc.tensor.matmul(
                            box['ps'], lhsT=ot[i2][:, tt * P:(tt + 1) * P],
                            rhs=wp[i2][:, ec * TG:(ec + 1) * TG],
                            start=(i2 == 0), stop=(i2 == 1))

                def drain():
                    # alternate the PSUM->SBUF cast and the store DMA across
                    # engines/queues so the drains pipeline 2-wide
                    ysb = ysbp.tile([P, TG], BF16, name="ysb", tag="ysb")
                    if alt:
                        nc.scalar.activation(
                            ysb, box['ps'], mybir.ActivationFunctionType.Copy)
                        nc.gpsimd.dma_start(
                            out=yp[tt * P:(tt + 1) * P,
                                   ec * TG:(ec + 1) * TG], in_=ysb)
                    else:
                        nc.vector.tensor_copy(ysb, box['ps'])
                        nc.sync.dma_start(
                            out=yp[tt * P:(tt + 1) * P,
                                   ec * TG:(ec + 1) * TG], in_=ysb)

                return [mm, drain]

            # ---- attention ----
            def attn_block(i, g, fq, pending=None, defer_av=False,
                           fast_recip=False):
                # causal: only s-chunks on/below the diagonal contribute.
                # `pending` is the previous block's deferred epilogue tail,
                # emitted after S(1) so its DMA-bounce waits never block this
                # block's DVE stream. `defer_av` emits all S's before any AV
                # (first block: the V chains feeding AV are still in fq).
                nj = NT if general_mask else 4 * g + 4
                av = [avp.tile([P, TG], F32, name="av", tag="av")
                      for _ in range(2)]
                pump_n = 2 if nj >= 8 else 1

                def emit_S(j):
                    st = stp.tile([P, 2 * TG], F32, name="st", tag="st")
                    for h in range(2):
                        nc.tensor.matmul(
                            st[:, h * TG:(h + 1) * TG],
                            lhsT=(kt[i][64 * h:64 * h + 64,
                                        j * P:(j + 1) * P]),
                            rhs=(qt[i][64 * h:64 * h + 64,
                                       g * TG:(g + 1) * TG]),
                            start=True, stop=True,
                            tile_position=(64 * h, 0))
                    r = j - 4 * g
                    if general_mask:
                        mk = mkp.tile([P, TG], F32, name="mk", tag="mk")
                        nc.sync.dma_start(
                            out=mk,
                            in_=maskT[j * P:(j + 1) * P, g * TG:(g + 1) * TG])
                        for h in range(2):
                            nc.vector.tensor_add(
                                st[:, h * TG:(h + 1) * TG],
                                st[:, h * TG:(h + 1) * TG], mk)
                    elif r >= 0:
                        for h in range(2):
                            sl = slice(h * TG + r * P, h * TG + (r + 1) * P)
                            nc.vector.tensor_add(st[:, sl], st[:, sl], band_t)
                    if apply_kbias:
                        for h in range(2):
                            nc.vector.tensor_scalar_add(
                                st[:, h * TG:(h + 1) * TG],
                                st[:, h * TG:(h + 1) * TG],
                                kbias_t[:, j:j + 1])
                    pt = ptp.tile([P, 2 * TG], BF16, name="pt", tag="pt")
                    nc.scalar.activation(pt, st, EXP, scale=1.0 / math.sqrt(D))
                    return pt

                def emit_AV(j, pt):
                    r = j - 4 * g
                    trim = r * P if (r > 0 and not general_mask) else 0
                    for h in range(2):
                        nc.tensor.matmul(
                            av[h][0:D + 1, trim:TG],
                            lhsT=(vaug[j][:, (2 * i + h) * (D + 1):
                                          (2 * i + h + 1) * (D + 1)]),
                            rhs=(pt[:, h * TG + trim:(h + 1) * TG]),
                            start=(j == 0), stop=(j == nj - 1),
                            skip_group_check=True)

                prev = None
                for j in range(nj):
                    pt = emit_S(j)
                    if j == 1 and pending is not None:
                        pending()
                        pending = None
                    if prev is not None and not defer_av:
                        emit_AV(*prev)
                    if not defer_av:
                        prev = (j, pt)
                    else:
                        prev = prev or []
                        prev.append((j, pt))
                    if j >= 1:
                        fq.pump(pump_n)
                if defer_av:
                    fq.flush()
                    for j, pt in prev:
                        emit_AV(j, pt)
                else:
                    emit_AV(*prev)

                # Epilogue part A: free the accumulator banks, launch the
                # denominator row into the DRAM transpose bounce.
                slot = i * NG + g
                asb = asbp.tile([D + 1, 2 * TG], BF16, name="asb", tag="asb")
                nc.vector.tensor_copy(asb[:, 0:TG], av[0][0:D + 1, :])
                nc.vector.tensor_copy(asb[:, TG:2 * TG], av[1][0:D + 1, :])

                def normalize(bc):
                    # upper-half (tm) first: its extra DMA hop into ot is on
                    # the critical path of the following projection
                    tm = tmpp.tile([P, TG], BF16, name="tm", tag="tm")
                    nc.vector.tensor_mul(tm[0:D, :], asb[0:D, TG:2 * TG],
                                         bc[0:D, TG:2 * TG])
                    nc.sync.dma_start(
                        out=ot[i][D:P, g * TG:(g + 1) * TG],
                        in_=tm[0:D, :])
                    nc.vector.tensor_mul(
                        ot[i][0:D, g * TG:(g + 1) * TG],
                        asb[0:D, 0:TG], bc[0:D, 0:TG])

                if fast_recip:
                    # tail block: skip the transpose bounce; reciprocal =
                    # exp(-ln(d)) on the (now idle) ScalarE, DMAs on the
                    # uncongested vector queue
                    rcl = rtp.tile([1, 2 * TG], F32, name="rcl", tag="rcl")
                    nc.scalar.activation(rcl, asb[D:D + 1, :],
                                         mybir.ActivationFunctionType.Ln)
                    rcx = rtp.tile([1, 2 * TG], BF16, name="rcx", tag="rcx")
                    nc.scalar.activation(rcx, rcl, EXP, scale=-1.0)
                    nc.scalar.dma_start(out=rcd_rcp[slot], in_=rcx)
                    bc = bcp.tile([P, 2 * TG], BF16, name="bc", tag="bc")
                    nc.scalar.dma_start(out=bc[0:D, :], in_=bass.AP(
                        tensor=rcd_rcp.tensor, offset=rcd_rcp[slot].offset,
                        ap=[[0, D], [1, 2 * TG]]))
                    normalize(bc)
                    return None

                nc.gpsimd.dma_start(out=rcd_raw[slot], in_=asb[D:D + 1, :])
                rt = rtp.tile([P, 8], BF16, name="rt", tag="rt")
                nc.gpsimd.dma_start(out=rt, in_=bass.AP(
                    tensor=rcd_raw.tensor, offset=rcd_raw[slot].offset,
                    ap=[[8, P], [1, 8]]))

                # Epilogue part B (deferred into the next block so the
                # bounce round-trips never stall this DVE/sync stream).
                def part_b():
                    rw = rtp.tile([P, 8], BF16, name="rw", tag="rw")
                    with nc.allow_low_precision(
                            reason="bf16 softmax denominators (~0.4% rel)"):
                        nc.vector.reciprocal(rw, rt)
                    nc.gpsimd.dma_start(out=bass.AP(
                        tensor=rcd_rcp.tensor, offset=rcd_rcp[slot].offset,
                        ap=[[8, P], [1, 8]]), in_=rw)
                    bc = bcp.tile([P, 2 * TG], BF16, name="bc", tag="bc")
                    nc.gpsimd.dma_start(out=bc[0:D, :], in_=bass.AP(
                        tensor=rcd_rcp.tensor, offset=rcd_rcp[slot].offset,
                        ap=[[0, D], [1, 2 * TG]]))
                    normalize(bc)

                return part_b

            # ---- emission schedule ----
            # Pair 0's first chains + V(0..3) run before its g=0 block; the
            # rest of QKV, pair-1 chains and the output projections are fed
            # through the filler queue into the attention blocks' spare PE
            # slots (attention is exp-paced on ScalarE).
            fq = FillerQueue()
            fq.add(qk_chain_units(0, 0))
            fq.flush()
            for j in range(4):
                fq.add(v_chain_units(j))
            pend = None
            for g in range(NG):
                if g < NG - 1:
                    fq.add(qk_chain_units(0, g + 1))
                    for j in range(4 * (g + 1), 4 * (g + 2)):
                        fq.add(v_chain_units(j))
                else:
                    fq.add(qk_chain_units(1, 0))
                pend = attn_block(0, g, fq, pending=pend, defer_av=(g == 0))
                fq.flush()
            for g in range(NG):
                if g < NG - 1:
                    fq.add(qk_chain_units(1, g + 1))
                pend = attn_block(1, g, fq, pending=pend,
                                  fast_recip=(g == NG - 1))
                fq.flush()
                for tt in range(4 * g, 4 * g + 4):
                    for ec in range(2):
                        fq.add(proj_units(tt, ec))
            if pend is not None:
                pend()
            fq.flush()

    nc.compile()
    return nc


@lru_cache(maxsize=4)
def _program(apply_kbias: bool, general_mask: bool) -> bass.Bass:
    return build_program(apply_kbias, general_mask)


def _host_prep(inputs):
    x = np.asarray(inputs["x"], np.float32)
    Wq = np.asarray(inputs["Wq"], np.float32)
    bq = np.asarray(inputs["bq"], np.float32)
    Wk = np.asarray(inputs["Wk"], np.float32)
    bk = np.asarray(inputs["bk"], np.float32)
    Wv = np.asarray(inputs["Wv"], np.float32)
    bv = np.asarray(inputs["bv"], np.float32)
    Wp = np.asarray(inputs["Wp"], np.float32)
    attn_mask = np.asarray(inputs["attn_mask"])
    valid = np.asarray(inputs["valid_input_mask"])

    tril = np.tril(np.ones((T, T), attn_mask.dtype))
    causal = all(np.array_equal(attn_mask[b], tril) for b in range(B))
    kbias_all = (valid.astype(np.float32) - 1.0) * 1e6  # [B, T]
    apply_kbias = bool((valid == 0).any())

    band = np.where(np.arange(P)[:, None] <= np.arange(P)[None, :],
                    np.float32(0.0), np.float32(NEG))

    in_maps = []
    for core in range(NCORES):
        b, hg = divmod(core, 4)
        sl = slice(hg * DD, (hg + 1) * DD)
        m = {
            "xT": np.ascontiguousarray(x[b].T).astype(ml_dtypes.bfloat16),
            "wqT": np.ascontiguousarray(Wq[sl, :].T).astype(ml_dtypes.bfloat16),
            "wkT": np.ascontiguousarray(Wk[sl, :].T).astype(ml_dtypes.bfloat16),
            "wvT": np.ascontiguousarray(Wv[sl, :].T).astype(ml_dtypes.bfloat16),
            "wpT": np.ascontiguousarray(Wp[:, sl].T).astype(ml_dtypes.bfloat16),
            "bqk": np.ascontiguousarray(
                np.stack([bq[sl][:P], bq[sl][P:], bk[sl][:P], bk[sl][P:]], 1)),
            "bv_sb": np.ascontiguousarray(np.tile(bv[sl], (P, 1))),
        }
        if apply_kbias:
            m["kbias"] = np.ascontiguousarray(kbias_all[b].reshape(NT, P).T)
        if not causal:
            m["maskT"] = np.ascontiguousarray(
                (attn_mask[b].T.astype(np.float32) - 1.0) * (-NEG))
        else:
            m["band"] = band
        in_maps.append(m)
    return in_maps, apply_kbias, causal


def _run(inputs, trace=False, trace_cores=None):
    global LAST_RESULTS
    in_maps, apply_kbias, causal = _host_prep(inputs)
    nc = _program(apply_kbias, not causal)
    res = bass_utils.run_bass_kernel_spmd(
        nc, in_maps, core_ids=list(range(NCORES)), trace=trace,
        trace_cores=trace_cores)
    LAST_RESULTS = res

    bp = np.asarray(inputs["bp"], np.float32)
    y = np.zeros((B, T, C), np.float32)
    for core in range(NCORES):
        y[core // 4] += np.asarray(res.results[core]["yp"], np.float32)
    y += bp[None, None, :]
    return y


def kernel(**inputs) -> np.ndarray:
    return _run(inputs)


# revision 15
# speedup vs baseline: 1.2339x; 1.0527x over previous
"""Causal self-attention (B=2, T=2048, C=1024, H=16) on 8 TRN2 NeuronCores.

Sharding: core = (batch b, head-group hg) with b in {0,1}, hg in {0..3};
each core computes Q/K/V projections and attention for its 4 heads on its
batch, plus the row-parallel slice of the output projection. The host sums
the 4 per-core partial projections per batch (bf16) and adds the output bias.

Device algorithm (all layouts transposed so softmax needs no on-chip
transposes):
  - x streamed t-group-major ([c, tg] tiles via one 3D-AP DMA per tg) so the
    K/Q projection chains start ~4us after launch; a burst of tiny warm-up
    matmuls releases the HAM clock throttle before the first real chain.
  - Q^T, K^T [dd, t] and V [t, dd] via bf16 matmul chains (contraction over C).
  - S^T[s, t]: 2 heads row-packed in the PE array (K=64 at row offsets 0/64).
  - exp on ScalarE straight out of PSUM (scale=1/sqrt(d) folded in); causal
    masking = one additive 128x128 band on diagonal blocks + trimming the
    AV matmul's moving range; softmax denominators from an all-ones column
    appended to V (M=65 matmul); normalization deferred to after AV.
  - softmax reciprocals: denominator row bounced through DRAM into a
    [128, 8] partition-major tile, one DVE reciprocal op (ScalarE runs
    exps only), bounced back and broadcast-read for the normalize muls.
  - Emission interleaves projection/QKV chains into the attention blocks'
    spare PE slots (attention is ScalarE-exp paced) so the PE never idles.
  - y_partial[t, e] bf16 out; host sums partials in f32.
"""

import math
from collections import deque
from functools import lru_cache

import ml_dtypes
import numpy as np

import concourse.bass as bass
import concourse.mybir as mybir
from concourse import bacc
import concourse.tile as tile
from concourse import bass_utils

F32 = mybir.dt.float32
BF16 = mybir.dt.bfloat16
EXP = mybir.ActivationFunctionType.Exp

B, T, C, H = 2, 2048, 1024, 16
NCORES = 8
NH = 4            # heads per core
D = C // H        # 64
DD = NH * D       # 256 channels per core
P = 128
TG = 512          # t-group width (matmul moving dim)
NG = T // TG      # 4
NT = T // P       # 16 s-chunks
CCH = C // P      # 8 contraction chunks
NEG = -8.0e6      # pre-scale additive mask; *0.125 = -1e6 like the reference

LAST_RESULTS = None  # BassKernelResults of the most recent run (for test.py)


class FillerQueue:
    """PE work units interleaved into the attention blocks' spare slots."""

    def __init__(self):
        self.q = deque()

    def add(self, units):
        self.q.extend(units)

    def add_front(self, units):
        for u in reversed(units):
            self.q.appendleft(u)

    def pump(self, n=1):
        for _ in range(n):
            if not self.q:
                return
            self.q.popleft()()

    def flush(self):
        while self.q:
            self.q.popleft()()


def build_program(apply_kbias: bool, general_mask: bool) -> bass.Bass:
    nc = bacc.Bacc("TRN2", target_bir_lowering=False, debug=False,
                   enable_asserts=False)

    xT = nc.dram_tensor("xT", [C, T], BF16, kind="ExternalInput").ap()
    wqT = nc.dram_tensor("wqT", [C, DD], BF16, kind="ExternalInput").ap()
    wkT = nc.dram_tensor("wkT", [C, DD], BF16, kind="ExternalInput").ap()
    wvT = nc.dram_tensor("wvT", [C, DD], BF16, kind="ExternalInput").ap()
    wpT = nc.dram_tensor("wpT", [DD, C], BF16, kind="ExternalInput").ap()
    bqk = nc.dram_tensor("bqk", [P, 4], F32, kind="ExternalInput").ap()
    bv_in = nc.dram_tensor("bv_sb", [P, DD], F32, kind="ExternalInput").ap()
    kbias_in = None
    if apply_kbias:
        kbias_in = nc.dram_tensor("kbias", [P, NT], F32, kind="ExternalInput").ap()
    band_in = maskT = None
    if general_mask:
        maskT = nc.dram_tensor("maskT", [T, T], F32, kind="ExternalInput").ap()
    else:
        band_in = nc.dram_tensor("band", [P, P], F32, kind="ExternalInput").ap()
    yp = nc.dram_tensor("yp", [T, C], BF16, kind="ExternalOutput").ap()
    # DRAM bounce buffers for the softmax denominators: raw rows land in
    # rcd_raw, get re-read [128, 8] partition-major (contiguous 8-elem lines),
    # reciprocated on DVE, written back t-major to rcd_rcp, then broadcast
    # across partitions (DMA from DRAM may use a 0-step partition dim).
    rcd_raw = nc.dram_tensor("rcd_raw", [2 * NG, 2 * TG], BF16, kind="Internal").ap()
    rcd_rcp = nc.dram_tensor("rcd_rcp", [2 * NG, 2 * TG], BF16, kind="Internal").ap()

    with tile.TileContext(nc) as tc:
        with tc.tile_pool(name="wts", bufs=1) as wts, \
             tc.tile_pool(name="xtp", bufs=1) as xtp, \
             tc.tile_pool(name="qkv", bufs=1) as qkv, \
             tc.tile_pool(name="otp", bufs=1) as otp, \
             tc.tile_pool(name="ptp", bufs=4) as ptp, \
             tc.tile_pool(name="asb", bufs=4) as asbp, \
             tc.tile_pool(name="rtp", bufs=2) as rtp, \
             tc.tile_pool(name="bcp", bufs=2) as bcp, \
             tc.tile_pool(name="tmp", bufs=3) as tmpp, \
             tc.tile_pool(name="ysb", bufs=6) as ysbp, \
             tc.tile_pool(name="mkp", bufs=2) as mkp, \
             tc.tile_pool(name="stp", bufs=2, space="PSUM") as stp, \
             tc.tile_pool(name="avp", bufs=2, space="PSUM") as avp, \
             tc.tile_pool(name="mmp", bufs=2, space="PSUM") as mmp:

            # Only Exp (and friends) are needed; preload so the act-table
            # DMA overlaps the input DMAs instead of stalling the first exp.
            from concourse.hw_specs import get_activation_tables
            tables = get_activation_tables(nc.m.arch)
            set_id = list(tables).index("natural_log_exp_and_others")
            nc.scalar.add_instruction(mybir.InstLoadActFuncSet(
                name=nc.get_next_instruction_name(), ins=[], outs=[],
                act_func_set_id=set_id))

            # ---- input DMAs ----
            bqk_t = wts.tile([P, 4], F32, name="bqk_t")
            nc.sync.dma_start(out=bqk_t, in_=bqk)

            # weights gathered c-major into single wide tiles (1 DMA each):
            # w[p, c*DD+q] = wT[c*P+p, q]
            def w_src(wT):
                return bass.AP(tensor=wT.tensor, offset=wT.offset,
                               ap=[[DD, P], [P * DD, CCH], [1, DD]])

            wkall = wts.tile([P, CCH * DD], BF16, name="wkall")
            wqall = wts.tile([P, CCH * DD], BF16, name="wqall")
            wvall = wts.tile([P, CCH * DD], BF16, name="wvall")
            # x gathered t-group-major: xtg[tg][p, c*TG+u] = xT[c*P+p, tg*TG+u]
            xtg = [xtp.tile([P, CCH * TG], BF16, name=f"xtg{t_}")
                   for t_ in range(NG)]

            def x_src(tg):
                return bass.AP(tensor=xT.tensor, offset=tg * TG,
                               ap=[[T, P], [P * T, CCH], [1, TG]])

            # xtg[0] streams on the gpsimd queue concurrently with the
            # weights on the sync queue; xtg[1..3] (3 MB, not needed until
            # ~t=16us) go at the END of the sync queue so they don't steal
            # DMA bandwidth from the first chains' weights.
            nc.sync.dma_start(out=wkall, in_=w_src(wkT))
            nc.gpsimd.dma_start(out=xtg[0], in_=x_src(0))
            nc.sync.dma_start(out=wqall, in_=w_src(wqT))
            nc.sync.dma_start(out=wvall, in_=w_src(wvT))
            bv_sb = wts.tile([P, DD], F32, name="bv_t")
            nc.sync.dma_start(out=bv_sb, in_=bv_in)
            if band_in is not None:
                band_t = wts.tile([P, P], F32, name="band_t")
                nc.sync.dma_start(out=band_t, in_=band_in)
            if kbias_in is not None:
                kbias_t = wts.tile([P, NT], F32, name="kbias_t")
                nc.sync.dma_start(out=kbias_t, in_=kbias_in)
            wp = [wts.tile([P, C], BF16, name=f"wp{i}") for i in range(2)]
            for i in range(2):
                nc.sync.dma_start(out=wp[i], in_=wpT[i * P:(i + 1) * P, :])
            for t_ in range(1, NG):
                nc.sync.dma_start(out=xtg[t_], in_=x_src(t_))

            qt = [qkv.tile([P, T], BF16, name=f"qt{i}") for i in range(2)]
            kt = [qkv.tile([P, T], BF16, name=f"kt{i}") for i in range(2)]
            vaug = [qkv.tile([P, NH * (D + 1)], BF16, name=f"vaug{j}")
                    for j in range(NT)]
            ot = [otp.tile([P, T], BF16, name=f"ot{i}") for i in range(2)]

            # ---- HAM warm-up: ~3us of tiny matmuls so the PE clock is at
            # 8/8 by the time the first projection chain lands.
            wps = mmp.tile([P, TG], F32, name="mm", tag="mm")
            for _ in range(100):
                nc.tensor.matmul(wps[0:1, 0:1], lhsT=bqk_t[:, 0:1],
                                 rhs=bqk_t[:, 0:1], start=True, stop=True)

            # ---- QKV chain units ----
            def wsl(wall, c, iw):
                return wall[:, c * DD + iw * P: c * DD + iw * P + P]

            def xsl(tg, c):
                return xtg[tg][:, c * TG:(c + 1) * TG]

            def xvsl(j, c):
                tg, u = divmod(j, NG)
                return xtg[tg][:, c * TG + u * P: c * TG + u * P + P]

            def qk_chain_units(iw, tg):
                """K then Q projection chain for dd-tile iw, t-group tg.
                Split into 2-matmul units + a trailing bias/drain unit (the
                drain lags its chain by one pump slot to avoid head-of-line
                blocking on the DVE queue)."""
                units = []
                for wall, dst, bcol in ((wqall, qt, iw), (wkall, kt, 2 + iw)):
                    box = {}

                    def mk_mm(c0, wall=wall, box=box):
                        def f():
                            if c0 == 0:
                                box['ps'] = mmp.tile([P, TG], F32, name="mm",
                                                     tag="mm")
                            for c in (c0, c0 + 1):
                                nc.tensor.matmul(
                                    box['ps'], lhsT=wsl(wall, c, iw),
                                    rhs=xsl(tg, c),
                                    start=(c == 0), stop=(c == CCH - 1))
                        return f

                    def mk_bias(dst=dst, bcol=bcol, box=box):
                        def f():
                            nc.vector.tensor_scalar_add(
                                dst[iw][:, tg * TG:(tg + 1) * TG], box['ps'],
                                bqk_t[:, bcol:bcol + 1])
                        return f

                    units += [mk_mm(0), mk_mm(2), mk_mm(4), mk_mm(6),
                              mk_bias()]
                return units

            def v_chain_units(j):
                box = {}

                def mk_mm(c0):
                    def f():
                        if c0 == 0:
                            box['ps'] = mmp.tile([P, TG], F32, name="mm",
                                                 tag="mm")
                        for c in range(c0, c0 + 4):
                            nc.tensor.matmul(
                                box['ps'][:, :DD], lhsT=xvsl(j, c),
                                rhs=wvall[:, c * DD:(c + 1) * DD],
                                start=(c == 0), stop=(c == CCH - 1))
                    return f

                def drain():
                    ps = box['ps']
                    vview = vaug[j].rearrange("p (h x) -> p h x", h=NH)
                    bvv = bv_sb.rearrange("p (h x) -> p h x", h=NH)
                    # ones column (softmax denominator row): in0*0 + 1
                    nc.vector.tensor_scalar(
                        vview[:, :, D:D + 1], bvv[:, :, 0:1], 0.0, 1.0,
                        mybir.AluOpType.mult, mybir.AluOpType.add)
                    nc.vector.tensor_add(
                        vview[:, :, 0:D],
                        ps[:, :DD].rearrange("p (h x) -> p h x", h=NH), bvv)

                return [mk_mm(0), mk_mm(4), drain]

            def proj_units(tt, ec):
                box = {}
                alt = (tt * 2 + ec) % 2

                def mm():
                    box['ps'] = mmp.tile([P, TG], F32, name="mm", tag="mm")
                    for i2 in range(2):
                        nc.tensor.matmul(
                            box['ps'], lhsT=ot[i2][:, tt * P:(tt + 1) * P],
                            rhs=wp[i2][:, ec * TG:(ec + 1) * TG],
                            start=(i2 == 0), stop=(i2 == 1))

                def drain():
                    # alternate the PSUM->SBUF cast and the store DMA across
                    # engines/queues so the drains pipeline 2-wide
                    ysb = ysbp.tile([P, TG], BF16, name="ysb", tag="ysb")
                    if alt:
                        nc.scalar.activation(
                            ysb, box['ps'], mybir.ActivationFunctionType.Copy)
                        nc.gpsimd.dma_start(
                            out=yp[tt * P:(tt + 1) * P,
                                   ec * TG:(ec + 1) * TG], in_=ysb)
                    else:
                        nc.vector.tensor_copy(ysb, box['ps'])
                        nc.sync.dma_start(
                            out=yp[tt * P:(tt + 1) * P,
                                   ec * TG:(ec + 1) * TG], in_=ysb)

                return [mm, drain]

            # ---- attention ----
            def attn_block(i, g, fq, pending=None, defer_av=False,
                           fast_recip=False, late_fq=None, tail_units=None):
                # causal: only s-chunks on/below the diagonal contribute.
                # `pending` is the previous block's deferred epilogue tail,
                # emitted after S(1) so its DMA-bounce waits never block this
                # block's DVE stream. `defer_av` emits all S's before any AV
                # (first block: the V chains feeding AV are still in fq).
                nj = NT if general_mask else 4 * g + 4
                av = [avp.tile([P, TG], F32, name="av", tag="av")
                      for _ in range(2)]
                pump_n = 3 if nj <= 4 else 2

                def pump(n):
                    for _ in range(n):
                        if fq.q:
                            fq.q.popleft()()
                        elif late_fq is not None and late_fq.q:
                            late_fq.q.popleft()()

                def emit_S(j):
                    st = stp.tile([P, 2 * TG], F32, name="st", tag="st")
                    for h in range(2):
                        nc.tensor.matmul(
                            st[:, h * TG:(h + 1) * TG],
                            lhsT=(kt[i][64 * h:64 * h + 64,
                                        j * P:(j + 1) * P]),
                            rhs=(qt[i][64 * h:64 * h + 64,
                                       g * TG:(g + 1) * TG]),
                            start=True, stop=True,
                            tile_position=(64 * h, 0))
                    r = j - 4 * g
                    if general_mask:
                        mk = mkp.tile([P, TG], F32, name="mk", tag="mk")
                        nc.sync.dma_start(
                            out=mk,
                            in_=maskT[j * P:(j + 1) * P, g * TG:(g + 1) * TG])
                        for h in range(2):
                            nc.vector.tensor_add(
                                st[:, h * TG:(h + 1) * TG],
                                st[:, h * TG:(h + 1) * TG], mk)
                    elif r >= 0:
                        for h in range(2):
                            sl = slice(h * TG + r * P, h * TG + (r + 1) * P)
                            nc.vector.tensor_add(st[:, sl], st[:, sl], band_t)
                    if apply_kbias:
                        for h in range(2):
                            nc.vector.tensor_scalar_add(
                                st[:, h * TG:(h + 1) * TG],
                                st[:, h * TG:(h + 1) * TG],
                                kbias_t[:, j:j + 1])
                    pt = ptp.tile([P, 2 * TG], BF16, name="pt", tag="pt")
                    nc.scalar.activation(pt, st, EXP, scale=1.0 / math.sqrt(D))
                    return pt

                def emit_AV(j, pt):
                    r = j - 4 * g
                    trim = r * P if (r > 0 and not general_mask) else 0
                    for h in range(2):
                        nc.tensor.matmul(
                            av[h][0:D + 1, trim:TG],
                            lhsT=(vaug[j][:, (2 * i + h) * (D + 1):
                                          (2 * i + h + 1) * (D + 1)]),
                            rhs=(pt[:, h * TG + trim:(h + 1) * TG]),
                            start=(j == 0), stop=(j == nj - 1),
                            skip_group_check=True)

                prev = None
                for j in range(nj):
                    pt = emit_S(j)
                    if j == 1 and pending is not None:
                        pending()
                        pending = None
                    if prev is not None and not defer_av:
                        if late_fq is not None and prev[0] == nj - 4:
                            late_fq.flush()
                        emit_AV(*prev)
                    if not defer_av:
                        prev = (j, pt)
                    else:
                        prev = prev or []
                        prev.append((j, pt))
                    if j >= 1:
                        pump(pump_n)
                if defer_av:
                    fq.flush()
                    for j, pt in prev:
                        emit_AV(j, pt)
                else:
                    if late_fq is not None and prev[0] == nj - 4:
                        late_fq.flush()
                    emit_AV(*prev)

                # Epilogue part A: free the accumulator banks, launch the
                # denominator row into the DRAM transpose bounce.
                slot = i * NG + g
                asb = asbp.tile([D + 1, 2 * TG], BF16, name="asb", tag="asb")
                nc.vector.tensor_copy(asb[:, 0:TG], av[0][0:D + 1, :])
                nc.vector.tensor_copy(asb[:, TG:2 * TG], av[1][0:D + 1, :])

                def normalize(bc):
                    # upper-half (tm) first: its extra DMA hop into ot is on
                    # the critical path of the following projection
                    tm = tmpp.tile([P, TG], BF16, name="tm", tag="tm")
                    nc.vector.tensor_mul(tm[0:D, :], asb[0:D, TG:2 * TG],
                                         bc[0:D, TG:2 * TG])
                    nc.sync.dma_start(
                        out=ot[i][D:P, g * TG:(g + 1) * TG],
                        in_=tm[0:D, :])
                    nc.vector.tensor_mul(
                        ot[i][0:D, g * TG:(g + 1) * TG],
                        asb[0:D, 0:TG], bc[0:D, 0:TG])

                if fast_recip:
                    # tail block: skip the transpose bounce; reciprocal =
                    # exp(-ln(d)) on the (now idle) ScalarE, DMAs on the
                    # uncongested vector queue
                    rcl = rtp.tile([1, 2 * TG], F32, name="rcl", tag="rcl")
                    nc.scalar.activation(rcl, asb[D:D + 1, :],
                                         mybir.ActivationFunctionType.Ln)
                    rcx = rtp.tile([1, 2 * TG], BF16, name="rcx", tag="rcx")
                    nc.scalar.activation(rcx, rcl, EXP, scale=-1.0)
                    nc.gpsimd.dma_start(out=rcd_rcp[slot], in_=rcx)
                    bc = bcp.tile([P, 2 * TG], BF16, name="bc", tag="bc")
                    nc.gpsimd.dma_start(out=bc[0:D, :], in_=bass.AP(
                        tensor=rcd_rcp.tensor, offset=rcd_rcp[slot].offset,
                        ap=[[0, D], [1, 2 * TG]]))
                    if tail_units:
                        for u in tail_units:
                            u()
                    normalize(bc)
                    return None

                nc.gpsimd.dma_start(out=rcd_raw[slot], in_=asb[D:D + 1, :])
                rt = rtp.tile([P, 8], BF16, name="rt", tag="rt")
                nc.gpsimd.dma_start(out=rt, in_=bass.AP(
                    tensor=rcd_raw.tensor, offset=rcd_raw[slot].offset,
                    ap=[[8, P], [1, 8]]))

                # Epilogue part B (deferred into the next block so the
                # bounce round-trips never stall this DVE/sync stream).
                def part_b():
                    rw = rtp.tile([P, 8], BF16, name="rw", tag="rw")
                    with nc.allow_low_precision(
                            reason="bf16 softmax denominators (~0.4% rel)"):
                        nc.vector.reciprocal(rw, rt)
                    nc.gpsimd.dma_start(out=bass.AP(
                        tensor=rcd_rcp.tensor, offset=rcd_rcp[slot].offset,
                        ap=[[8, P], [1, 8]]), in_=rw)
                    bc = bcp.tile([P, 2 * TG], BF16, name="bc", tag="bc")
                    nc.gpsimd.dma_start(out=bc[0:D, :], in_=bass.AP(
                        tensor=rcd_rcp.tensor, offset=rcd_rcp[slot].offset,
                        ap=[[0, D], [1, 2 * TG]]))
                    normalize(bc)

                return part_b

            # ---- emission schedule ----
            # Pair 0's first chains + V(0..3) run before its g=0 block; the
            # rest of QKV, pair-1 chains and the output projections are fed
            # through the filler queue into the attention blocks' spare PE
            # slots (attention is exp-paced on ScalarE).
            fq = FillerQueue()
            late = FillerQueue()
            fq.add(qk_chain_units(0, 0))
            fq.flush()
            for j in range(4):
                fq.add(v_chain_units(j))
            pend = None
            for g in range(NG):
                if g < NG - 1:
                    fq.add(qk_chain_units(0, g + 1))
                    for j in range(4 * (g + 1), 4 * (g + 2)):
                        late.add(v_chain_units(j))
                else:
                    fq.add(qk_chain_units(1, 0))
                pend = attn_block(0, g, fq, pending=pend, defer_av=(g == 0),
                                  late_fq=(None if g == 0 else late))
                fq.flush()
            for g in range(NG):
                tail_units = None
                if g < NG - 1:
                    fq.add_front(qk_chain_units(1, g + 1))
                else:
                    tail_units = [fq.q.pop()
                                  for _ in range(min(6, len(fq.q)))][::-1]
                pend = attn_block(1, g, fq, pending=pend,
                                  fast_recip=(g == NG - 1),
                                  tail_units=tail_units)
                fq.flush()
                for tt in range(4 * g, 4 * g + 4):
                    for ec in range(2):
                        fq.add(proj_units(tt, ec))
            if pend is not None:
                pend()
            fq.flush()

    nc.compile()
    return nc


@lru_cache(maxsize=4)
def _program(apply_kbias: bool, general_mask: bool) -> bass.Bass:
    return build_program(apply_kbias, general_mask)


def _host_prep(inputs):
    x = np.asarray(inputs["x"], np.float32)
    Wq = np.asarray(inputs["Wq"], np.float32)
    bq = np.asarray(inputs["bq"], np.float32)
    Wk = np.asarray(inputs["Wk"], np.float32)
    bk = np.asarray(inputs["bk"], np.float32)
    Wv = np.asarray(inputs["Wv"], np.float32)
    bv = np.asarray(inputs["bv"], np.float32)
    Wp = np.asarray(inputs["Wp"], np.float32)
    attn_mask = np.asarray(inputs["attn_mask"])
    valid = np.asarray(inputs["valid_input_mask"])

    tril = np.tril(np.ones((T, T), attn_mask.dtype))
    causal = all(np.array_equal(attn_mask[b], tril) for b in range(B))
    kbias_all = (valid.astype(np.float32) - 1.0) * 1e6  # [B, T]
    apply_kbias = bool((valid == 0).any())

    band = np.where(np.arange(P)[:, None] <= np.arange(P)[None, :],
                    np.float32(0.0), np.float32(NEG))

    in_maps = []
    for core in range(NCORES):
        b, hg = divmod(core, 4)
        sl = slice(hg * DD, (hg + 1) * DD)
        m = {
            "xT": np.ascontiguousarray(x[b].T).astype(ml_dtypes.bfloat16),
            "wqT": np.ascontiguousarray(Wq[sl, :].T).astype(ml_dtypes.bfloat16),
            "wkT": np.ascontiguousarray(Wk[sl, :].T).astype(ml_dtypes.bfloat16),
            "wvT": np.ascontiguousarray(Wv[sl, :].T).astype(ml_dtypes.bfloat16),
            "wpT": np.ascontiguousarray(Wp[:, sl].T).astype(ml_dtypes.bfloat16),
            "bqk": np.ascontiguousarray(
                np.stack([bq[sl][:P], bq[sl][P:], bk[sl][:P], bk[sl][P:]], 1)),
            "bv_sb": np.ascontiguousarray(np.tile(bv[sl], (P, 1))),
        }
        if apply_kbias:
            m["kbias"] = np.ascontiguousarray(kbias_all[b].reshape(NT, P).T)
        if not causal:
            m["maskT"] = np.ascontiguousarray(
                (attn_mask[b].T.astype(np.float32) - 1.0) * (-NEG))
        else:
            m["band"] = band
        in_maps.append(m)
    return in_maps, apply_kbias, causal


def _run(inputs, trace=False, trace_cores=None):
    global LAST_RESULTS
    in_maps, apply_kbias, causal = _host_prep(inputs)
    nc = _program(apply_kbias, not causal)
    res = bass_utils.run_bass_kernel_spmd(
        nc, in_maps, core_ids=list(range(NCORES)), trace=trace,
        trace_cores=trace_cores)
    LAST_RESULTS = res

    bp = np.asarray(inputs["bp"], np.float32)
    y = np.zeros((B, T, C), np.float32)
    for core in range(NCORES):
        y[core // 4] += np.asarray(res.results[core]["yp"], np.float32)
    y += bp[None, None, :]
    return y


def kernel(**inputs) -> np.ndarray:
    return _run(inputs)


# revision 17
# speedup vs baseline: 1.2774x; 1.0353x over previous
"""Causal self-attention (B=2, T=2048, C=1024, H=16) on 8 TRN2 NeuronCores.

Sharding: core = (batch b, head-group hg) with b in {0,1}, hg in {0..3};
each core computes Q/K/V projections and attention for its 4 heads on its
batch, plus the row-parallel slice of the output projection. The host sums
the 4 per-core partial projections per batch (bf16) and adds the output bias.

Device algorithm (all layouts transposed so softmax needs no on-chip
transposes):
  - x streamed t-group-major ([c, tg] tiles via one 3D-AP DMA per tg) so the
    K/Q projection chains start ~4us after launch; a burst of tiny warm-up
    matmuls releases the HAM clock throttle before the first real chain.
  - Q^T, K^T [dd, t] and V [t, dd] via bf16 matmul chains (contraction over C).
  - S^T[s, t]: 2 heads row-packed in the PE array (K=64 at row offsets 0/64).
  - exp on ScalarE straight out of PSUM (scale=1/sqrt(d) folded in); causal
    masking = one additive 128x128 band on diagonal blocks + trimming the
    AV matmul's moving range; softmax denominators from an all-ones column
    appended to V (M=65 matmul); normalization deferred to after AV.
  - softmax reciprocals: denominator row bounced through DRAM into a
    [128, 8] partition-major tile, one DVE reciprocal op (ScalarE runs
    exps only), bounced back and broadcast-read for the normalize muls.
  - Emission interleaves projection/QKV chains into the attention blocks'
    spare PE slots (attention is ScalarE-exp paced) so the PE never idles.
  - y_partial[t, e] bf16 out; host sums partials in f32.
"""

import math
from collections import deque
from functools import lru_cache

import ml_dtypes
import numpy as np

import concourse.bass as bass
import concourse.mybir as mybir
from concourse import bacc
import concourse.tile as tile
from concourse import bass_utils

F32 = mybir.dt.float32
BF16 = mybir.dt.bfloat16
EXP = mybir.ActivationFunctionType.Exp

B, T, C, H = 2, 2048, 1024, 16
NCORES = 8
NH = 4            # heads per core
D = C // H        # 64
DD = NH * D       # 256 channels per core
P = 128
TG = 512          # t-group width (matmul moving dim)
NG = T // TG      # 4
NT = T // P       # 16 s-chunks
CCH = C // P      # 8 contraction chunks
NEG = -8.0e6      # pre-scale additive mask; *0.125 = -1e6 like the reference

LAST_RESULTS = None  # BassKernelResults of the most recent run (for test.py)


class FillerQueue:
    """PE work units interleaved into the attention blocks' spare slots."""

    def __init__(self):
        self.q = deque()

    def add(self, units):
        self.q.extend(units)

    def add_front(self, units):
        for u in reversed(units):
            self.q.appendleft(u)

    def pump(self, n=1):
        for _ in range(n):
            if not self.q:
                return
            self.q.popleft()()

    def flush(self):
        while self.q:
            self.q.popleft()()


def build_program(apply_kbias: bool, general_mask: bool) -> bass.Bass:
    nc = bacc.Bacc("TRN2", target_bir_lowering=False, debug=False,
                   enable_asserts=False)

    xT = nc.dram_tensor("xT", [C, T], BF16, kind="ExternalInput").ap()
    wqT = nc.dram_tensor("wqT", [C, DD], BF16, kind="ExternalInput").ap()
    wkT = nc.dram_tensor("wkT", [C, DD], BF16, kind="ExternalInput").ap()
    wvT = nc.dram_tensor("wvT", [C, DD], BF16, kind="ExternalInput").ap()
    wpT = nc.dram_tensor("wpT", [DD, C], BF16, kind="ExternalInput").ap()
    bqk = nc.dram_tensor("bqk", [P, 4], F32, kind="ExternalInput").ap()
    bv_in = nc.dram_tensor("bv_sb", [P, DD], F32, kind="ExternalInput").ap()
    kbias_in = None
    if apply_kbias:
        kbias_in = nc.dram_tensor("kbias", [P, NT], F32, kind="ExternalInput").ap()
    band_in = maskT = None
    if general_mask:
        maskT = nc.dram_tensor("maskT", [T, T], F32, kind="ExternalInput").ap()
    else:
        band_in = nc.dram_tensor("band", [P, P], F32, kind="ExternalInput").ap()
    yp = nc.dram_tensor("yp", [T, C], BF16, kind="ExternalOutput").ap()
    # DRAM bounce buffers for the softmax denominators: raw rows land in
    # rcd_raw, get re-read [128, 8] partition-major (contiguous 8-elem lines),
    # reciprocated on DVE, written back t-major to rcd_rcp, then broadcast
    # across partitions (DMA from DRAM may use a 0-step partition dim).
    rcd_raw = nc.dram_tensor("rcd_raw", [2 * NG, 2 * TG], BF16, kind="Internal").ap()
    rcd_rcp = nc.dram_tensor("rcd_rcp", [2 * NG, 2 * TG], BF16, kind="Internal").ap()

    with tile.TileContext(nc) as tc:
        with tc.tile_pool(name="wts", bufs=1) as wts, \
             tc.tile_pool(name="xtp", bufs=1) as xtp, \
             tc.tile_pool(name="qkv", bufs=1) as qkv, \
             tc.tile_pool(name="otp", bufs=1) as otp, \
             tc.tile_pool(name="ptp", bufs=4) as ptp, \
             tc.tile_pool(name="asb", bufs=4) as asbp, \
             tc.tile_pool(name="rtp", bufs=2) as rtp, \
             tc.tile_pool(name="bcp", bufs=2) as bcp, \
             tc.tile_pool(name="tmp", bufs=3) as tmpp, \
             tc.tile_pool(name="ysb", bufs=6) as ysbp, \
             tc.tile_pool(name="mkp", bufs=2) as mkp, \
             tc.tile_pool(name="stp", bufs=2, space="PSUM") as stp, \
             tc.tile_pool(name="avp", bufs=2, space="PSUM") as avp, \
             tc.tile_pool(name="mmp", bufs=2, space="PSUM") as mmp:

            # Only Exp (and friends) are needed; preload so the act-table
            # DMA overlaps the input DMAs instead of stalling the first exp.
            from concourse.hw_specs import get_activation_tables
            tables = get_activation_tables(nc.m.arch)
            set_id = list(tables).index("natural_log_exp_and_others")
            nc.scalar.add_instruction(mybir.InstLoadActFuncSet(
                name=nc.get_next_instruction_name(), ins=[], outs=[],
                act_func_set_id=set_id))

            # ---- input DMAs ----
            bqk_t = wts.tile([P, 4], F32, name="bqk_t")
            nc.sync.dma_start(out=bqk_t, in_=bqk)

            # weights gathered c-major into single wide tiles (1 DMA each):
            # w[p, c*DD+q] = wT[c*P+p, q]
            def w_src(wT):
                return bass.AP(tensor=wT.tensor, offset=wT.offset,
                               ap=[[DD, P], [P * DD, CCH], [1, DD]])

            # wk/wq split into per-pair halves so pair-0's chains only wait
            # on 0.5 MB of weights at startup
            wkh = [wts.tile([P, CCH * P], BF16, name=f"wkh{i}") for i in range(2)]
            wqh = [wts.tile([P, CCH * P], BF16, name=f"wqh{i}") for i in range(2)]
            wvall = wts.tile([P, CCH * DD], BF16, name="wvall")

            def wh_src(wT, iw):
                return bass.AP(tensor=wT.tensor, offset=wT.offset + iw * P,
                               ap=[[DD, P], [P * DD, CCH], [1, P]])
            # x gathered t-group-major: xtg[tg][p, c*TG+u] = xT[c*P+p, tg*TG+u]
            xtg = [xtp.tile([P, CCH * TG], BF16, name=f"xtg{t_}")
                   for t_ in range(NG)]

            def x_src(tg):
                return bass.AP(tensor=xT.tensor, offset=tg * TG,
                               ap=[[T, P], [P * T, CCH], [1, TG]])

            # xtg[0] streams on the gpsimd queue concurrently with the
            # weights on the sync queue; xtg[1..3] (3 MB, not needed until
            # ~t=16us) go at the END of the sync queue so they don't steal
            # DMA bandwidth from the first chains' weights.
            nc.sync.dma_start(out=wkh[0], in_=wh_src(wkT, 0))
            nc.gpsimd.dma_start(out=xtg[0], in_=x_src(0))
            nc.sync.dma_start(out=wqh[0], in_=wh_src(wqT, 0))
            nc.sync.dma_start(out=wvall, in_=w_src(wvT))
            nc.sync.dma_start(out=wkh[1], in_=wh_src(wkT, 1))
            nc.sync.dma_start(out=wqh[1], in_=wh_src(wqT, 1))
            bv_sb = wts.tile([P, DD], F32, name="bv_t")
            nc.sync.dma_start(out=bv_sb, in_=bv_in)
            if band_in is not None:
                band_t = wts.tile([P, P], F32, name="band_t")
                nc.sync.dma_start(out=band_t, in_=band_in)
            if kbias_in is not None:
                kbias_t = wts.tile([P, NT], F32, name="kbias_t")
                nc.sync.dma_start(out=kbias_t, in_=kbias_in)
            wp = [wts.tile([P, C], BF16, name=f"wp{i}") for i in range(2)]
            for i in range(2):
                nc.sync.dma_start(out=wp[i], in_=wpT[i * P:(i + 1) * P, :])
            for t_ in range(1, NG):
                nc.sync.dma_start(out=xtg[t_], in_=x_src(t_))

            qt = [qkv.tile([P, T], BF16, name=f"qt{i}") for i in range(2)]
            kt = [qkv.tile([P, T], BF16, name=f"kt{i}") for i in range(2)]
            vaug = [qkv.tile([P, NH * (D + 1)], BF16, name=f"vaug{j}")
                    for j in range(NT)]
            ot = [otp.tile([P, T], BF16, name=f"ot{i}") for i in range(2)]

            # ---- HAM warm-up: ~3us of tiny matmuls so the PE clock is at
            # 8/8 by the time the first projection chain lands.
            wps = mmp.tile([P, TG], F32, name="mm", tag="mm")
            for _ in range(100):
                nc.tensor.matmul(wps[0:1, 0:1], lhsT=bqk_t[:, 0:1],
                                 rhs=bqk_t[:, 0:1], start=True, stop=True)

            # ---- QKV chain units ----
            def wsl(whalves, c, iw):
                return whalves[iw][:, c * P:(c + 1) * P]

            def xsl(tg, c):
                return xtg[tg][:, c * TG:(c + 1) * TG]

            def xvsl(j, c):
                tg, u = divmod(j, NG)
                return xtg[tg][:, c * TG + u * P: c * TG + u * P + P]

            def qk_chain_units(iw, tg):
                """K then Q projection chain for dd-tile iw, t-group tg.
                Split into 2-matmul units + a trailing bias/drain unit (the
                drain lags its chain by one pump slot to avoid head-of-line
                blocking on the DVE queue)."""
                units = []
                for wall, dst, bcol in ((wqh, qt, iw), (wkh, kt, 2 + iw)):
                    box = {}

                    def mk_mm(c0, wall=wall, box=box):
                        def f():
                            if c0 == 0:
                                box['ps'] = mmp.tile([P, TG], F32, name="mm",
                                                     tag="mm")
                            for c in (c0, c0 + 1):
                                nc.tensor.matmul(
                                    box['ps'], lhsT=wsl(wall, c, iw),
                                    rhs=xsl(tg, c),
                                    start=(c == 0), stop=(c == CCH - 1))
                        return f

                    def mk_bias(dst=dst, bcol=bcol, box=box):
                        def f():
                            nc.vector.tensor_scalar_add(
                                dst[iw][:, tg * TG:(tg + 1) * TG], box['ps'],
                                bqk_t[:, bcol:bcol + 1])
                        return f

                    units += [mk_mm(0), mk_mm(2), mk_mm(4), mk_mm(6),
                              mk_bias()]
                return units

            def v_chain_units(j):
                box = {}

                def mk_mm(c0):
                    def f():
                        if c0 == 0:
                            box['ps'] = mmp.tile([P, TG], F32, name="mm",
                                                 tag="mm")
                        for c in range(c0, c0 + 4):
                            nc.tensor.matmul(
                                box['ps'][:, :DD], lhsT=xvsl(j, c),
                                rhs=wvall[:, c * DD:(c + 1) * DD],
                                start=(c == 0), stop=(c == CCH - 1))
                    return f

                def drain():
                    ps = box['ps']
                    vview = vaug[j].rearrange("p (h x) -> p h x", h=NH)
                    bvv = bv_sb.rearrange("p (h x) -> p h x", h=NH)
                    # ones column (softmax denominator row): in0*0 + 1
                    nc.vector.tensor_scalar(
                        vview[:, :, D:D + 1], bvv[:, :, 0:1], 0.0, 1.0,
                        mybir.AluOpType.mult, mybir.AluOpType.add)
                    nc.vector.tensor_add(
                        vview[:, :, 0:D],
                        ps[:, :DD].rearrange("p (h x) -> p h x", h=NH), bvv)

                return [mk_mm(0), mk_mm(4), drain]

            def proj_units(tt, ec):
                box = {}
                alt = (tt * 2 + ec) % 2

                def mm():
                    box['ps'] = mmp.tile([P, TG], F32, name="mm", tag="mm")
                    for i2 in range(2):
                        nc.tensor.matmul(
                            box['ps'], lhsT=ot[i2][:, tt * P:(tt + 1) * P],
                            rhs=wp[i2][:, ec * TG:(ec + 1) * TG],
                            start=(i2 == 0), stop=(i2 == 1))

                def drain():
                    # alternate the PSUM->SBUF cast and the store DMA across
                    # engines/queues so the drains pipeline 2-wide
                    ysb = ysbp.tile([P, TG], BF16, name="ysb", tag="ysb")
                    if alt:
                        nc.scalar.activation(
                            ysb, box['ps'], mybir.ActivationFunctionType.Copy)
                        nc.gpsimd.dma_start(
                            out=yp[tt * P:(tt + 1) * P,
                                   ec * TG:(ec + 1) * TG], in_=ysb)
                    else:
                        nc.vector.tensor_copy(ysb, box['ps'])
                        nc.sync.dma_start(
                            out=yp[tt * P:(tt + 1) * P,
                                   ec * TG:(ec + 1) * TG], in_=ysb)

                return [mm, drain]

            # ---- attention ----
            def attn_block(i, g, fq, pending=None, defer_av=False,
                           fast_recip=False, late_fq=None, tail_units=None):
                # causal: only s-chunks on/below the diagonal contribute.
                # `pending` is the previous block's deferred epilogue tail,
                # emitted after S(1) so its DMA-bounce waits never block this
                # block's DVE stream. `defer_av` emits all S's before any AV
                # (first block: the V chains feeding AV are still in fq).
                nj = NT if general_mask else 4 * g + 4
                av = [avp.tile([P, TG], F32, name="av", tag="av")
                      for _ in range(2)]
                pump_n = 2 if nj == 8 else 3

                def pump(n):
                    for _ in range(n):
                        if late_fq is not None and late_fq.q:
                            late_fq.q.popleft()()
                        elif fq.q:
                            fq.q.popleft()()

                def emit_S(j):
                    st = stp.tile([P, 2 * TG], F32, name="st", tag="st")
                    for h in range(2):
                        nc.tensor.matmul(
                            st[:, h * TG:(h + 1) * TG],
                            lhsT=(kt[i][64 * h:64 * h + 64,
                                        j * P:(j + 1) * P]),
                            rhs=(qt[i][64 * h:64 * h + 64,
                                       g * TG:(g + 1) * TG]),
                            start=True, stop=True,
                            tile_position=(64 * h, 0))
                    r = j - 4 * g
                    if general_mask:
                        mk = mkp.tile([P, TG], F32, name="mk", tag="mk")
                        nc.sync.dma_start(
                            out=mk,
                            in_=maskT[j * P:(j + 1) * P, g * TG:(g + 1) * TG])
                        for h in range(2):
                            nc.vector.tensor_add(
                                st[:, h * TG:(h + 1) * TG],
                                st[:, h * TG:(h + 1) * TG], mk)
                    elif r >= 0:
                        for h in range(2):
                            sl = slice(h * TG + r * P, h * TG + (r + 1) * P)
                            nc.vector.tensor_add(st[:, sl], st[:, sl], band_t)
                    if apply_kbias:
                        for h in range(2):
                            nc.vector.tensor_scalar_add(
                                st[:, h * TG:(h + 1) * TG],
                                st[:, h * TG:(h + 1) * TG],
                                kbias_t[:, j:j + 1])
                    pt = ptp.tile([P, 2 * TG], BF16, name="pt", tag="pt")
                    nc.scalar.activation(pt, st, EXP, scale=1.0 / math.sqrt(D))
                    return pt

                def emit_AV(j, pt):
                    r = j - 4 * g
                    trim = r * P if (r > 0 and not general_mask) else 0
                    for h in range(2):
                        nc.tensor.matmul(
                            av[h][0:D + 1, trim:TG],
                            lhsT=(vaug[j][:, (2 * i + h) * (D + 1):
                                          (2 * i + h + 1) * (D + 1)]),
                            rhs=(pt[:, h * TG + trim:(h + 1) * TG]),
                            start=(j == 0), stop=(j == nj - 1),
                            skip_group_check=True)

                prev = None
                for j in range(nj):
                    pt = emit_S(j)
                    if j == 1 and pending is not None:
                        pending()
                        pending = None
                    if prev is not None and not defer_av:
                        if late_fq is not None and prev[0] == nj - 4:
                            late_fq.flush()
                        emit_AV(*prev)
                    if not defer_av:
                        prev = (j, pt)
                    else:
                        prev = prev or []
                        prev.append((j, pt))
                    if j >= 1:
                        pump(pump_n)
                if defer_av:
                    fq.flush()
                    for j, pt in prev:
                        emit_AV(j, pt)
                else:
                    if late_fq is not None and prev[0] == nj - 4:
                        late_fq.flush()
                    emit_AV(*prev)

                # Epilogue part A: free the accumulator banks, launch the
                # denominator row into the DRAM transpose bounce.
                slot = i * NG + g
                asb = asbp.tile([D + 1, 2 * TG], BF16, name="asb", tag="asb")
                nc.vector.tensor_copy(asb[:, 0:TG], av[0][0:D + 1, :])
                nc.vector.tensor_copy(asb[:, TG:2 * TG], av[1][0:D + 1, :])

                def normalize(bc):
                    # upper-half (tm) first: its extra DMA hop into ot is on
                    # the critical path of the following projection
                    tm = tmpp.tile([P, TG], BF16, name="tm", tag="tm")
                    nc.vector.tensor_mul(tm[0:D, :], asb[0:D, TG:2 * TG],
                                         bc[0:D, TG:2 * TG])
                    nc.sync.dma_start(
                        out=ot[i][D:P, g * TG:(g + 1) * TG],
                        in_=tm[0:D, :])
                    nc.vector.tensor_mul(
                        ot[i][0:D, g * TG:(g + 1) * TG],
                        asb[0:D, 0:TG], bc[0:D, 0:TG])

                if fast_recip:
                    # Tail block. Reciprocal = exp(-ln(d)) on the (now idle)
                    # ScalarE from a dedicated early copy of the denominator
                    # rows; meanwhile the final projection's accumulation
                    # chains OPEN with their ot[0]-half (ready since phase 1)
                    # across all 8 free PSUM banks, and CLOSE with the
                    # ot[1]-half after the normalize - so the PE works
                    # through the reciprocal round-trip instead of idling.
                    rdc = rtp.tile([1, 2 * TG], BF16, name="rdc", tag="rdc")
                    nc.vector.tensor_copy(rdc[:, 0:TG], av[0][D:D + 1, :])
                    nc.vector.tensor_copy(rdc[:, TG:2 * TG], av[1][D:D + 1, :])
                    rcl = rtp.tile([1, 2 * TG], F32, name="rcl", tag="rcl")
                    nc.scalar.activation(rcl, rdc,
                                         mybir.ActivationFunctionType.Ln)
                    rcx = rtp.tile([1, 2 * TG], BF16, name="rcx", tag="rcx")
                    nc.scalar.activation(rcx, rcl, EXP, scale=-1.0)
                    nc.gpsimd.dma_start(out=rcd_rcp[slot], in_=rcx)
                    bc = bcp.tile([P, 2 * TG], BF16, name="bc", tag="bc")
                    nc.gpsimd.dma_start(out=bc[0:D, :], in_=bass.AP(
                        tensor=rcd_rcp.tensor, offset=rcd_rcp[slot].offset,
                        ap=[[0, D], [1, 2 * TG]]))
                    tp = []
                    st_t = None
                    for bi, (tt, ec) in enumerate(
                            (t_, e_) for t_ in range(4 * g, 4 * g + 4)
                            for e_ in range(2)):
                        if bi < 4:
                            if bi % 2 == 0:
                                st_t = stp.tile([P, 2 * TG], F32, name="st",
                                                tag="st")
                            ps = st_t[:, (bi % 2) * TG:(bi % 2 + 1) * TG]
                        elif bi < 6:
                            ps = avp.tile([P, TG], F32, name="av", tag="av")
                        else:
                            ps = mmp.tile([P, TG], F32, name="mm", tag="mm")
                        nc.tensor.matmul(
                            ps, lhsT=ot[0][:, tt * P:(tt + 1) * P],
                            rhs=wp[0][:, ec * TG:(ec + 1) * TG],
                            start=True, stop=False, skip_group_check=True)
                        tp.append((tt, ec, ps))
                    normalize(bc)
                    for bi, (tt, ec, ps) in enumerate(tp):
                        nc.tensor.matmul(
                            ps, lhsT=ot[1][:, tt * P:(tt + 1) * P],
                            rhs=wp[1][:, ec * TG:(ec + 1) * TG],
                            start=False, stop=True, skip_group_check=True)
                        ysb = ysbp.tile([P, TG], BF16, name="ysb", tag="ysb")
                        if bi % 2:
                            nc.scalar.activation(
                                ysb, ps, mybir.ActivationFunctionType.Copy)
                            nc.gpsimd.dma_start(
                                out=yp[tt * P:(tt + 1) * P,
                                       ec * TG:(ec + 1) * TG], in_=ysb)
                        else:
                            nc.vector.tensor_copy(ysb, ps)
                            nc.sync.dma_start(
                                out=yp[tt * P:(tt + 1) * P,
                                       ec * TG:(ec + 1) * TG], in_=ysb)
                    return None

                nc.gpsimd.dma_start(out=rcd_raw[slot], in_=asb[D:D + 1, :])
                rt = rtp.tile([P, 8], BF16, name="rt", tag="rt")
                nc.gpsimd.dma_start(out=rt, in_=bass.AP(
                    tensor=rcd_raw.tensor, offset=rcd_raw[slot].offset,
                    ap=[[8, P], [1, 8]]))

                # Epilogue part B (deferred into the next block so the
                # bounce round-trips never stall this DVE/sync stream).
                def part_b():
                    rw = rtp.tile([P, 8], BF16, name="rw", tag="rw")
                    with nc.allow_low_precision(
                            reason="bf16 softmax denominators (~0.4% rel)"):
                        nc.vector.reciprocal(rw, rt)
                    nc.gpsimd.dma_start(out=bass.AP(
                        tensor=rcd_rcp.tensor, offset=rcd_rcp[slot].offset,
                        ap=[[8, P], [1, 8]]), in_=rw)
                    bc = bcp.tile([P, 2 * TG], BF16, name="bc", tag="bc")
                    nc.gpsimd.dma_start(out=bc[0:D, :], in_=bass.AP(
                        tensor=rcd_rcp.tensor, offset=rcd_rcp[slot].offset,
                        ap=[[0, D], [1, 2 * TG]]))
                    normalize(bc)

                return part_b

            # ---- emission schedule ----
            # Pair 0's first chains + V(0..3) run before its g=0 block; the
            # rest of QKV, pair-1 chains and the output projections are fed
            # through the filler queue into the attention blocks' spare PE
            # slots (attention is exp-paced on ScalarE).
            fq = FillerQueue()
            fq.add(qk_chain_units(0, 0))
            fq.flush()
            for j in range(4):
                fq.add(v_chain_units(j))
            pend = None
            cur_late = None
            for g in range(NG):
                if g < NG - 1:
                    fq.add(qk_chain_units(0, g + 1))
                    nxt_late = FillerQueue()
                    for j in range(4 * (g + 1), 4 * (g + 2)):
                        nxt_late.add(v_chain_units(j))
                else:
                    fq.add(qk_chain_units(1, 0))
                    nxt_late = None
                pend = attn_block(0, g, fq, pending=pend, defer_av=(g == 0),
                                  late_fq=cur_late)
                fq.flush()
                cur_late = nxt_late
            for g in range(NG):
                if g < NG - 1:
                    fq.add_front(qk_chain_units(1, g + 1))
                pend = attn_block(1, g, fq, pending=pend,
                                  fast_recip=(g == NG - 1))
                fq.flush()
                if g < NG - 1:
                    for tt in range(4 * g, 4 * g + 4):
                        for ec in range(2):
                            fq.add(proj_units(tt, ec))
            if pend is not None:
                pend()
            fq.flush()

    nc.compile()
    return nc


@lru_cache(maxsize=4)
def _program(apply_kbias: bool, general_mask: bool) -> bass.Bass:
    return build_program(apply_kbias, general_mask)


def _host_prep(inputs):
    x = np.asarray(inputs["x"], np.float32)
    Wq = np.asarray(inputs["Wq"], np.float32)
    bq = np.asarray(inputs["bq"], np.float32)
    Wk = np.asarray(inputs["Wk"], np.float32)
    bk = np.asarray(inputs["bk"], np.float32)
    Wv = np.asarray(inputs["Wv"], np.float32)
    bv = np.asarray(inputs["bv"], np.float32)
    Wp = np.asarray(inputs["Wp"], np.float32)
    attn_mask = np.asarray(inputs["attn_mask"])
    valid = np.asarray(inputs["valid_input_mask"])

    tril = np.tril(np.ones((T, T), attn_mask.dtype))
    causal = all(np.array_equal(attn_mask[b], tril) for b in range(B))
    kbias_all = (valid.astype(np.float32) - 1.0) * 1e6  # [B, T]
    apply_kbias = bool((valid == 0).any())

    band = np.where(np.arange(P)[:, None] <= np.arange(P)[None, :],
                    np.float32(0.0), np.float32(NEG))

    in_maps = []
    for core in range(NCORES):
        b, hg = divmod(core, 4)
        sl = slice(hg * DD, (hg + 1) * DD)
        m = {
            "xT": np.ascontiguousarray(x[b].T).astype(ml_dtypes.bfloat16),
            "wqT": np.ascontiguousarray(Wq[sl, :].T).astype(ml_dtypes.bfloat16),
            "wkT": np.ascontiguousarray(Wk[sl, :].T).astype(ml_dtypes.bfloat16),
            "wvT": np.ascontiguousarray(Wv[sl, :].T).astype(ml_dtypes.bfloat16),
            "wpT": np.ascontiguousarray(Wp[:, sl].T).astype(ml_dtypes.bfloat16),
            "bqk": np.ascontiguousarray(
                np.stack([bq[sl][:P], bq[sl][P:], bk[sl][:P], bk[sl][P:]], 1)),
            "bv_sb": np.ascontiguousarray(np.tile(bv[sl], (P, 1))),
        }
        if apply_kbias:
            m["kbias"] = np.ascontiguousarray(kbias_all[b].reshape(NT, P).T)
        if not causal:
            m["maskT"] = np.ascontiguousarray(
                (attn_mask[b].T.astype(np.float32) - 1.0) * (-NEG))
        else:
            m["band"] = band
        in_maps.append(m)
    return in_maps, apply_kbias, causal


def _run(inputs, trace=False, trace_cores=None):
    global LAST_RESULTS
    in_maps, apply_kbias, causal = _host_prep(inputs)
    nc = _program(apply_kbias, not causal)
    res = bass_utils.run_bass_kernel_spmd(
        nc, in_maps, core_ids=list(range(NCORES)), trace=trace,
        trace_cores=trace_cores)
    LAST_RESULTS = res

    bp = np.asarray(inputs["bp"], np.float32)
    y = np.zeros((B, T, C), np.float32)
    for core in range(NCORES):
        y[core // 4] += np.asarray(res.results[core]["yp"], np.float32)
    y += bp[None, None, :]
    return y


def kernel(**inputs) -> np.ndarray:
    return _run(inputs)
